# revision 10
# baseline (speedup 1.0000x reference)
"""Trainium2 Bass kernel v3 for nn_AC_Filter_PreNorm_Net (causal attention +
product-network Euler).

Self-contained: accepts FULL inputs, shards batch over 8 NeuronCores, returns
FULL output.

v3 changes over v2 (251us baseline):
  - Euler 8-factor product as a tensor_tensor tree (tt1 PSUM pair-mult ->
    SBUF bf16, then 4x-mode bf16 tts) instead of tensor_reduce (no DVE perf
    modes) + Pool cross-copy.  Pool msh copy (59.6us) eliminated.
  - Euler state written to a per-step ring of stateT tiles; output DMA'd
    directly from stateT (E-major bf16) and transposed on host.  The 128
    outT transposes + 32 scalar copies are gone.
  - Scores narrowed to the causal trapezoid: diagonal k-tiles only compute
    q >= off*128; single shared [128,128] triangle mask (Pool), zero-fill
    memsets on Pool.
  - pov ring bufs=2 so b1's AV no longer waits b0's tail.
"""
import sys
sys.path.insert(0, "/opt/trn_rl_repo")
import numpy as np
import concourse.bass as bass
import concourse.tile as tile
import bass_rust
from concourse import mybir
from concourse.bass_utils import run_bass_kernel_spmd

F32 = mybir.dt.float32
BF16 = mybir.dt.bfloat16
AF = mybir.ActivationFunctionType
MULT = mybir.AluOpType.mult
ADD = mybir.AluOpType.add

B, L, D = 16, 2048, 63
E = D + 1            # 64
W1 = 8
F_LEN = 4
DT = 0.01
EPS = 1e-5
NCORES = 8
BPC = B // NCORES    # batches per core = 2
NT = L // 128        # l-tiles per batch = 16
NC4 = 4              # q-chunks of 512


def _split_multiwaits(nc):
    """walrus rejects >1 sync wait per instruction; hoist extras onto
    preceding same-engine NoOps."""
    n_added = 0
    for fn in nc.m.functions:
        for bb in fn.blocks:
            insts = list(bb.instructions)
            out = []
            changed = False
            for inst in insts:
                si = inst.sync_info
                if si is not None and si.on_wait is not None and len(si.on_wait) > 1:
                    waits = list(si.on_wait)
                    for w in waits[:-1]:
                        nop = mybir.InstNoOp(
                            name=f"{inst.name}-wsp{n_added}", ins=[], outs=[]
                        )
                        n_added += 1
                        nop.engine = inst.engine
                        nop.sync_info = bass_rust.SyncInfo(on_wait=[w], on_update=[])
                        out.append(nop)
                    si.on_wait = [waits[-1]]
                    changed = True
                out.append(inst)
            if changed:
                bb.instructions = out
    return n_added


def _build_nc(split=True):
    nc = bass.Bass()
    dp = nc.declare_dram_parameter
    xt_e = dp("xt", [BPC, E, L], BF16, isOutput=False)       # host-pretransposed
    wqkt_e = dp("wqkt", [E, 128], BF16, isOutput=False)      # lhsT: [e_in, q|k]
    wut_e = dp("wut", [E, E], BF16, isOutput=False)          # rhs: [e_in, e_out]
    wall_e = dp("wall", [E, 4 * 128], BF16, isOutput=False)  # grouped, DT*s folded
    tri_e = dp("tri", [128, 128], BF16, isOutput=False)      # causal triangle
    ident_e = dp("ident", [128, 128], BF16, isOutput=False)
    # E-major per-step state dump; host transposes to [L, F_LEN*D]
    out_e = dp("out", [BPC, NC4, F_LEN, D, 512], BF16, isOutput=True)

    with tile.TileContext(nc) as tc:
        with (
            tc.tile_pool(name="consts", bufs=1) as cp,
            tc.tile_pool(name="big", bufs=2) as bp,
            tc.tile_pool(name="chk", bufs=2) as chp,
            tc.tile_pool(name="ps", bufs=1, space="PSUM") as psP,
        ):
            # ---- first const (HAM burst + qk gate on it), then inputs ----
            wqkt = cp.tile([E, 128], BF16)
            nc.sync.dma_start(out=wqkt[:], in_=wqkt_e[:])
            xts = []
            for b in range(BPC):
                xt = bp.tile([E, L], BF16, tag="xt")
                nc.sync.dma_start(out=xt[:], in_=xt_e[b])
                xts.append(xt)
            wut = cp.tile([E, E], BF16)
            nc.sync.dma_start(out=wut[:], in_=wut_e[:])
            wall = cp.tile([E, 4 * 128], BF16)
            nc.sync.dma_start(out=wall[:], in_=wall_e[:])
            tri = cp.tile([128, 128], BF16)
            nc.sync.dma_start(out=tri[:], in_=tri_e[:])
            ident = cp.tile([128, 128], BF16)
            nc.sync.dma_start(out=ident[:], in_=ident_e[:])

            # ---- HAM warm-up burst ----
            # The PE clock gate passes 4/8 pulses (1.2 GHz) until one
            # fully-busy free-running 3.41us window elapses, then 8/8
            # (2.4 GHz) until a fully-idle one.  Real work always has small
            # gaps; burn ~8us of back-to-back matmuls (overlapping the xt
            # DMA wait) to cover a full window at any alignment.
            for i in range(72):
                hw_ = psP.tile([128, 1024], F32, tag="hh", bufs=2, name="hh")
                nc.tensor.matmul(
                    hw_[:, 0:128], wqkt[:], wqkt[:, 0:128],
                    start=True, stop=True)

            # activation-table preload (overlaps DMA wait)
            _scr = cp.tile([64, 4], BF16, name="actwarm")
            nc.vector.memset(_scr[:], 0.0)
            nc.scalar.activation(_scr[:], _scr[:], AF.Exp)

            st = {}   # persistent per-batch tiles

            # combined per-step stateT ring [64, b0|b1 512 each]: slot t =
            # state after t Euler steps (slot 0 written by the attention
            # tails each chunk).  Row 63 is the pinned ones-row: slots 1..4
            # only get rows 0:63 written by the Euler add, so set once
            # (whole tile: engine ops need a 0/32/64/96 start partition).
            states = []
            for t in range(F_LEN + 1):
                s_t = cp.tile([E, BPC * 512], BF16, name=f"state{t}")
                states.append(s_t)
                if t > 0:
                    nc.gpsimd.memset(s_t[:], 1.0)

            # ================= attention thunk lists =================
            def attn_thunks(b, c):
                """List of closures emitting attention for (b, c), in
                queue-safe order."""
                ops = []
                nki = 4 * c + 4
                npair = nki // 2

                if c == 0:
                    def ldx(b=b):
                        xt = xts[b]
                        qT = bp.tile([E, L], BF16, tag="qT")
                        kT = bp.tile([E, L], BF16, tag="kT")
                        u_aug = bp.tile([128, NT * (E + 1)], BF16, tag="u_aug")
                        st[b] = {"xt": xt, "qT": qT, "kT": kT, "u_aug": u_aug}
                    ops.append(ldx)

                    def qk(cp_, b=b):
                        s_ = st[b]
                        ps = psP.tile([128, 512], F32, tag="sc", bufs=2,
                                      name="ps")
                        nc.tensor.matmul(
                            ps[:], wqkt[:],
                            s_["xt"][:, cp_ * 512:(cp_ + 1) * 512],
                            start=True, stop=True)
                        nc.vector.tensor_copy(
                            s_["qT"][:, cp_ * 512:(cp_ + 1) * 512], ps[0:E, :])
                        nc.scalar.copy(
                            s_["kT"][:, cp_ * 512:(cp_ + 1) * 512], ps[64:128, :])
                    for cp_ in range(4):
                        ops.append(lambda b=b, cp_=cp_: qk(cp_, b))

                    def uproj(uh, b=b):
                        s_ = st[b]
                        ps = psP.tile([128, 512], F32, tag="pov", bufs=2,
                                      name="ups")
                        for j in range(8):
                            lt = uh * 8 + j
                            nc.tensor.matmul(
                                ps[:, j * 64:(j + 1) * 64],
                                s_["xt"][:, lt * 128:(lt + 1) * 128], wut[:],
                                start=True, stop=True)
                        ua = s_["u_aug"][:].rearrange("p (n e1) -> p n e1", e1=E + 1)
                        if uh == 0:
                            nc.vector.memset(ua[:, :, E:E + 1], 1.0)
                        nc.scalar.copy(
                            ua[:, uh * 8:(uh + 1) * 8, 0:E],
                            ps[:].rearrange("p (n e) -> p n e", e=E))
                    ops.append(lambda b=b: uproj(0, b))
                    ops.append(lambda b=b: uproj(1, b))

                exps_tiles = {}

                def scone(ki, b=b, c=c):
                    s_ = st[b]
                    off = ki - 4 * c
                    q0 = off * 128 if off > 0 else 0
                    ps = psP.tile([128, 512], F32, tag="sc", bufs=2, name="ps")
                    nc.tensor.matmul(
                        ps[:, q0:512],
                        s_["kT"][:, ki * 128:(ki + 1) * 128],
                        s_["qT"][:, c * 512 + q0:(c + 1) * 512],
                        start=True, stop=True)
                    exps = chp.tile([128, 512], BF16, tag="exps", bufs=8,
                                    name="exps")
                    nc.scalar.activation(exps[:, q0:512], ps[:, q0:512], AF.Exp)
                    if off >= 0:
                        # causal triangle on the diagonal 128-col group
                        nc.gpsimd.tensor_tensor(
                            exps[:, q0:q0 + 128], exps[:, q0:q0 + 128],
                            tri[:], MULT)
                    exps_tiles[ki] = exps

                def av(ki, b=b, c=c, nki=nki):
                    if ki == 0:
                        pov = psP.tile([65, 512], F32, tag="pov", bufs=2,
                                       name="pov")
                        st[(b, c, "pov")] = pov
                    pov = st[(b, c, "pov")]
                    eh = exps_tiles.pop(ki)
                    ua = st[b]["u_aug"][:].rearrange(
                        "p (n e1) -> p n e1", e1=E + 1)
                    # diagonal blocks contribute only to q >= off*128
                    off = ki - 4 * c
                    q0 = off * 128 if off > 0 else 0
                    nc.tensor.matmul(
                        pov[:, q0:512], ua[:, ki, :], eh[:, q0:512],
                        start=(ki == 0), stop=(ki == nki - 1),
                        skip_group_check=True)

                # interleave: scores run ~3 blocks ahead of av
                sq = list(range(nki))
                aq = list(range(nki))
                while sq or aq:
                    if sq:
                        ki = sq.pop(0)
                        ops.append(lambda ki=ki: scone(ki))
                    done = nki - len(sq)
                    if aq and (not sq or aq[0] <= done - 3):
                        ki = aq.pop(0)
                        ops.append(lambda ki=ki: av(ki))

                def tail(b=b, c=c):
                    pov = st.pop((b, c, "pov"))
                    o_un = chp.tile([65, 512], BF16, tag="o_un")
                    nc.scalar.copy(o_un[:], pov[:])
                    # stride 66 keeps each PSUM transpose write 4B-aligned
                    tr = psP.tile([128, 4 * 66], BF16, tag="sc", bufs=2)
                    tr_v = tr[:].rearrange("p (n e1) -> p n e1", e1=66)
                    for j in range(4):
                        nc.tensor.transpose(
                            tr_v[:, j, 0:65],
                            o_un[:, j * 128:(j + 1) * 128],
                            ident[0:65, 0:65])
                    rden = chp.tile([128, 4], BF16, tag="rden")
                    with nc.allow_low_precision(reason="bf16 recip of softmax denom, 0.4%"):
                        nc.vector.reciprocal(rden[:], tr_v[:, :, 64])
                    # state_l columns PERMUTED: col p = state e=p+1 (p<63), col 63 = ones
                    state_l = chp.tile([128, 4 * E], BF16, tag="state_l")
                    sl = state_l[:].rearrange("p (n e) -> p n e", e=E)
                    nc.vector.memset(sl[:, :, D:E], 1.0)
                    nc.vector.tensor_tensor(
                        sl[:, :, 0:D], tr_v[:, :, 1:E],
                        rden[:, :, None].to_broadcast([128, 4, D]), MULT)
                    stT_ps = psP.tile([64, 512], BF16, tag="sc", bufs=2)
                    for j in range(4):
                        nc.tensor.transpose(
                            stT_ps[:, j * 128:(j + 1) * 128],
                            sl[:, j, :], ident[:])
                    # stateT slot 0, this batch's half (incl ones row)
                    nc.scalar.copy(
                        states[0][:, b * 512:(b + 1) * 512], stT_ps[:])
                ops.append(tail)
                return ops

            # ================= euler emission =================
            def euler_unit(c, t, bsel=None, msh_on_scalar=True,
                           scalar_quarters=0):
                """One Euler step (states[t] -> states[t+1] + DMA rows 0:63).
                bsel=None: both batches batched; bsel=b: that batch's half
                only (used for t=0 so euler starts right after b's tail)."""
                s_in = states[t]
                s_out = states[t + 1]
                bs = range(BPC) if bsel is None else [bsel]
                w = len(bs) * 512
                x0 = 0 if bsel is None else bsel * 512
                # per (b, half): 4 group matmuls; h layout [128, g*256+l];
                # 4-group product reduce (DVE can read only ONE PSUM operand
                # per instruction, so a tt-tree on h is illegal).
                mm = chp.tile([128, BPC * 512], BF16, tag="mm", bufs=2,
                              name="mm")
                nq = 0
                for b in bs:
                    for half in range(2):
                        h = psP.tile([128, 1024], F32, tag="hh", name="hh",
                                     bufs=2)
                        q0 = b * 512 + half * 256
                        for g in range(4):
                            nc.tensor.matmul(
                                h[:, g * 256:(g + 1) * 256],
                                wall[:, g * 128:(g + 1) * 128],
                                s_in[:, q0:q0 + 256],
                                start=True, stop=True)
                        if nq < scalar_quarters:
                            # scalar moves h to SBUF; DVE tree at 2x beats
                            # the perf-mode-less 1x reduce
                            h_sb = chp.tile([128, 1024], BF16, tag="hsb",
                                            bufs=2, name="hsb")
                            nc.scalar.copy(h_sb[:], h[:])
                            hv = h_sb[:].rearrange(
                                "p (x g l) -> p x g l", x=2, g=2)
                            m1 = chp.tile([128, 512], BF16, tag="m1",
                                          bufs=2, name="m1")
                            m1v = m1[:].rearrange("p (x l) -> p x l", x=2)
                            nc.vector.tensor_tensor(
                                m1v[:], hv[:, :, 0, :], hv[:, :, 1, :], MULT)
                            nc.vector.tensor_tensor(
                                mm[:, q0:q0 + 256],
                                m1v[:, 0, :], m1v[:, 1, :], MULT)
                            nq += 1
                        else:
                            nc.vector.tensor_reduce(
                                mm[:, q0:q0 + 256],
                                h[:].rearrange("p (g l) -> p l g", l=256),
                                mybir.AxisListType.X, MULT)
                # cross-half product + state add (batched over selected bs)
                msh = chp.tile([63, BPC * 512], BF16, tag="msh", bufs=2,
                               name="msh")
                if msh_on_scalar:
                    nc.scalar.copy(msh[:, x0:x0 + w], mm[64:64 + D, x0:x0 + w])
                else:
                    nc.vector.tensor_copy(
                        msh[:, x0:x0 + w], mm[64:64 + D, x0:x0 + w])
                nc.vector.tensor_tensor(
                    msh[:, x0:x0 + w], msh[:, x0:x0 + w],
                    mm[0:D, x0:x0 + w], MULT)
                nc.vector.tensor_tensor(
                    s_out[0:D, x0:x0 + w], s_in[0:D, x0:x0 + w],
                    msh[:, x0:x0 + w], ADD)
                # DMA: src iterates (d, b, q); dst [b, c, t, d, q]
                nc.sync.dma_start(
                    out=bass.AP(
                        tensor=out_e,
                        offset=(c * F_LEN + t) * D * 512
                        + (0 if bsel is None else bsel * NC4 * F_LEN * D * 512),
                        ap=[[512, D], [NC4 * F_LEN * D * 512, len(bs)],
                            [1, 512]]),
                    in_=s_out[0:D, x0:x0 + w].rearrange(
                        "p (b q) -> p b q", b=len(bs)))

            # ================= schedule =================
            def zip_merge(a, b):
                out = []
                for i in range(max(len(a), len(b))):
                    if i < len(a):
                        out.append(a[i])
                    if i < len(b):
                        out.append(b[i])
                return out

            # startup: attention for chunk 0, batches zipped so neither
            # engine chain (sc ring / scalar exp) starves the PE queue
            for op in zip_merge(attn_thunks(0, 0), attn_thunks(1, 0)):
                op()

            # global attention queue with per-chunk completion deadlines:
            # drain as LATE as possible while attn(c) still fully precedes
            # euler(c) -- keeps DVE (euler) busy early, spreads the PE-heavy
            # attention over the whole run.  bounds0[cc] = b0's ops done.
            GQ = []
            bounds = {}
            bounds0 = {}
            for cc in range(1, NC4):
                a0 = attn_thunks(0, cc)
                a1 = attn_thunks(1, cc)
                GQ += zip_merge(a0, a1)
                bounds0[cc] = len(GQ) - (1 if len(a1) >= len(a0) else 0)
                bounds[cc] = len(GQ)
            drained = 0
            units_done = 0
            for c in range(NC4):
                last = c == NC4 - 1
                if c >= 1:
                    # b0's attention done -> start b0's first euler step
                    while drained < bounds0[c]:
                        GQ[drained]()
                        drained += 1
                    euler_unit(c, 0, bsel=0,
                               scalar_quarters=(2 if last else 1 if c == 2 else 0))
                    while drained < bounds[c]:
                        GQ[drained]()
                        drained += 1
                    euler_unit(c, 0, bsel=1,
                               scalar_quarters=(2 if last else 1 if c == 2 else 0))
                else:
                    euler_unit(c, 0)
                units_done += 1
                for t in range(1, F_LEN):
                    euler_unit(c, t,
                               scalar_quarters=(2 if last else 1 if c == 2 else 0))
                    units_done += 1
                    need = 0
                    for cc in range(c + 1, NC4):
                        ub = cc * F_LEN - units_done
                        rem = bounds[cc] - drained
                        if rem <= 0:
                            continue
                        need = max(need, rem if ub <= 0 else -(-rem // ub))
                    for _ in range(need):
                        if drained < len(GQ):
                            GQ[drained]()
                            drained += 1
            while drained < len(GQ):
                GQ[drained]()
                drained += 1

    if split:
        _split_multiwaits(nc)
    return nc


_NC_CACHE = None


def _get_nc():
    global _NC_CACHE
    if _NC_CACHE is None:
        _NC_CACHE = _build_nc()
    return _NC_CACHE


def host_prep(in_proj_w, out_proj_w, Wg, Mg, bg, sigma):
    """Host-side weight prep (shared with the sim test)."""
    bf = mybir.dt.np(BF16)
    s = sigma + EPS
    inv_s_aug = np.concatenate([[1.0], 1.0 / s]).astype(np.float32)
    Win_f = in_proj_w * inv_s_aug[None, :]
    scale = 1.0 / np.sqrt(np.float32(E))
    Wq = Win_f[0:E] * scale
    Wk = Win_f[E:2 * E]
    Wv = Win_f[2 * E:3 * E]

    # fused V*out_proj with physical-units fold
    Wu = out_proj_w @ Wv
    Wu[0, :] = 0.0
    Wu[1:, :] = Wu[1:, :] * s[:, None]

    wqkt = np.concatenate([Wq, Wk], axis=0).T.astype(bf)       # [64, 128]
    wut = Wu.T.astype(bf)                                      # [64, 64]

    # wall grouped: group g is [64, 128]: col d <-> factor 2g ch d, col 64+d
    # <-> factor 2g+1 ch d; cols 63/127 dummy-zero. Rows permuted to the
    # stateT layout (states e=1..63 at partitions 0..62, ones-row at 63).
    Wgm = Wg * Mg
    wall = np.zeros((E, 4 * 128), np.float32)
    for g in range(4):
        for f in range(2):
            j = 2 * g + f
            col = g * 128 + f * 64 + np.arange(D)
            wall[D, col] = Wgm[:, j, 0] + bg[:, j]            # ones-row coeff
            wall[0:D, col] = (Wgm[:, j, 1:] / s[None, :]).T   # states rows
    wall[:, 0:63] *= DT * s[None, :]   # fold DT and s_d into factor 0
    wall = wall.astype(bf)

    kv = np.arange(128)[:, None]
    q = np.arange(128)[None, :]
    tri = (kv <= q).astype(np.float32).astype(bf)
    ident = np.eye(128).astype(bf)
    return wqkt, wut, wall, tri, ident


def kernel(t, inputs, in_proj_w, in_proj_b, out_proj_w, out_proj_b,
           Wg, Mg, bg, sigma):
    inputs = np.asarray(inputs, np.float32)
    in_proj_w = np.asarray(in_proj_w, np.float32)
    in_proj_b = np.asarray(in_proj_b, np.float32)
    out_proj_w = np.asarray(out_proj_w, np.float32)
    out_proj_b = np.asarray(out_proj_b, np.float32)
    Wg = np.asarray(Wg, np.float32)
    Mg = np.asarray(Mg, np.float32)
    bg = np.asarray(bg, np.float32)
    sigma = np.asarray(sigma, np.float32)
    bf = mybir.dt.np(BF16)
    assert np.all(in_proj_b == 0) and np.all(out_proj_b == 0)

    wqkt, wut, wall, tri, ident = host_prep(
        in_proj_w, out_proj_w, Wg, Mg, bg, sigma)

    xt_all = np.ascontiguousarray(
        inputs.reshape(NCORES, BPC, L, E).transpose(0, 1, 3, 2)).astype(bf)

    in_maps = []
    for i in range(NCORES):
        in_maps.append({
            "xt": xt_all[i], "wqkt": wqkt, "wut": wut,
            "wall": wall, "tri": tri, "ident": ident,
        })

    nc = _get_nc()
    res = run_bass_kernel_spmd(nc, in_maps, core_ids=list(range(NCORES)))
    global LAST_RESULTS
    LAST_RESULTS = res
    # out: [BPC, NC4, F_LEN, D, 512] bf16, E-major -> [B, L, F_LEN*D] f32
    outs = []
    for i in range(NCORES):
        o = np.asarray(res.results[i]["out"]).astype(np.float32)
        # [BPC, c, t, d, q] -> [BPC, c, q, t, d]
        o = o.transpose(0, 1, 4, 2, 3).reshape(BPC, L, F_LEN * D)
        outs.append(o)
    return np.ascontiguousarray(np.concatenate(outs, axis=0))


LAST_RESULTS = None


# revision 11
# speedup vs baseline: 1.0237x; 1.0237x over previous
"""Trainium2 Bass kernel v3 for nn_AC_Filter_PreNorm_Net (causal attention +
product-network Euler).

Self-contained: accepts FULL inputs, shards batch over 8 NeuronCores, returns
FULL output.

v3 changes over v2 (251us baseline):
  - Euler 8-factor product as a tensor_tensor tree (tt1 PSUM pair-mult ->
    SBUF bf16, then 4x-mode bf16 tts) instead of tensor_reduce (no DVE perf
    modes) + Pool cross-copy.  Pool msh copy (59.6us) eliminated.
  - Euler state written to a per-step ring of stateT tiles; output DMA'd
    directly from stateT (E-major bf16) and transposed on host.  The 128
    outT transposes + 32 scalar copies are gone.
  - Scores narrowed to the causal trapezoid: diagonal k-tiles only compute
    q >= off*128; single shared [128,128] triangle mask (Pool), zero-fill
    memsets on Pool.
  - pov ring bufs=2 so b1's AV no longer waits b0's tail.
"""
import sys
sys.path.insert(0, "/opt/trn_rl_repo")
import numpy as np
import concourse.bass as bass
import concourse.tile as tile
import bass_rust
from concourse import mybir
from concourse.bass_utils import run_bass_kernel_spmd

F32 = mybir.dt.float32
BF16 = mybir.dt.bfloat16
AF = mybir.ActivationFunctionType
MULT = mybir.AluOpType.mult
ADD = mybir.AluOpType.add

B, L, D = 16, 2048, 63
E = D + 1            # 64
W1 = 8
F_LEN = 4
DT = 0.01
EPS = 1e-5
NCORES = 8
BPC = B // NCORES    # batches per core = 2
NT = L // 128        # l-tiles per batch = 16
NC4 = 4              # q-chunks of 512


def _split_multiwaits(nc):
    """walrus rejects >1 sync wait per instruction; hoist extras onto
    preceding same-engine NoOps."""
    n_added = 0
    for fn in nc.m.functions:
        for bb in fn.blocks:
            insts = list(bb.instructions)
            out = []
            changed = False
            for inst in insts:
                si = inst.sync_info
                if si is not None and si.on_wait is not None and len(si.on_wait) > 1:
                    waits = list(si.on_wait)
                    for w in waits[:-1]:
                        nop = mybir.InstNoOp(
                            name=f"{inst.name}-wsp{n_added}", ins=[], outs=[]
                        )
                        n_added += 1
                        nop.engine = inst.engine
                        nop.sync_info = bass_rust.SyncInfo(on_wait=[w], on_update=[])
                        out.append(nop)
                    si.on_wait = [waits[-1]]
                    changed = True
                out.append(inst)
            if changed:
                bb.instructions = out
    return n_added


def _build_nc(split=True):
    nc = bass.Bass()
    dp = nc.declare_dram_parameter
    xt_e = dp("xt", [BPC, E, L], BF16, isOutput=False)       # host-pretransposed
    wqkt_e = dp("wqkt", [E, 128], BF16, isOutput=False)      # lhsT: [e_in, q|k]
    wut_e = dp("wut", [E, E], BF16, isOutput=False)          # rhs: [e_in, e_out]
    wall_e = dp("wall", [E, 4 * 128], BF16, isOutput=False)  # grouped, DT*s folded
    tri_e = dp("tri", [128, 128], BF16, isOutput=False)      # causal triangle
    ident_e = dp("ident", [128, 128], BF16, isOutput=False)
    # E-major per-step state dump; host transposes to [L, F_LEN*D]
    out_e = dp("out", [BPC, NC4, F_LEN, D, 512], BF16, isOutput=True)

    with tile.TileContext(nc) as tc:
        with (
            tc.tile_pool(name="consts", bufs=1) as cp,
            tc.tile_pool(name="big", bufs=2) as bp,
            tc.tile_pool(name="chk", bufs=2) as chp,
            tc.tile_pool(name="ps", bufs=1, space="PSUM") as psP,
        ):
            # ---- first const (HAM burst + qk gate on it), then inputs ----
            wqkt = cp.tile([E, 128], BF16)
            nc.sync.dma_start(out=wqkt[:], in_=wqkt_e[:])
            xts = []
            for b in range(BPC):
                xt = bp.tile([E, L], BF16, tag="xt")
                nc.sync.dma_start(out=xt[:], in_=xt_e[b])
                xts.append(xt)
            wut = cp.tile([E, E], BF16)
            nc.sync.dma_start(out=wut[:], in_=wut_e[:])
            wall = cp.tile([E, 4 * 128], BF16)
            nc.sync.dma_start(out=wall[:], in_=wall_e[:])
            tri = cp.tile([128, 128], BF16)
            nc.sync.dma_start(out=tri[:], in_=tri_e[:])
            ident = cp.tile([128, 128], BF16)
            nc.sync.dma_start(out=ident[:], in_=ident_e[:])

            # activation-table preload (overlaps DMA wait)
            _scr = cp.tile([64, 4], BF16, name="actwarm")
            nc.vector.memset(_scr[:], 0.0)
            nc.scalar.activation(_scr[:], _scr[:], AF.Exp)

            st = {}   # persistent per-batch tiles

            # combined per-step stateT ring [64, b0|b1 512 each]: slot t =
            # state after t Euler steps (slot 0 written by the attention
            # tails each chunk).  Row 63 is the pinned ones-row: slots 1..4
            # only get rows 0:63 written by the Euler add, so set once
            # (whole tile: engine ops need a 0/32/64/96 start partition).
            states = []
            for t in range(F_LEN + 1):
                s_t = cp.tile([E, BPC * 512], BF16, name=f"state{t}")
                states.append(s_t)
                if t > 0:
                    nc.gpsimd.memset(s_t[:], 1.0)

            # ================= attention thunk lists =================
            def attn_thunks(b, c):
                """List of closures emitting attention for (b, c), in
                queue-safe order."""
                ops = []
                nki = 4 * c + 4
                npair = nki // 2

                if c == 0:
                    def ldx(b=b):
                        xt = xts[b]
                        qT = bp.tile([E, L], BF16, tag="qT")
                        kT = bp.tile([E, L], BF16, tag="kT")
                        u_aug = bp.tile([128, NT * (E + 1)], BF16, tag="u_aug")
                        st[b] = {"xt": xt, "qT": qT, "kT": kT, "u_aug": u_aug}
                    ops.append(ldx)

                    def qk(cp_, b=b):
                        s_ = st[b]
                        ps = psP.tile([128, 512], F32, tag="sc", bufs=2,
                                      name="ps")
                        nc.tensor.matmul(
                            ps[:], wqkt[:],
                            s_["xt"][:, cp_ * 512:(cp_ + 1) * 512],
                            start=True, stop=True)
                        nc.vector.tensor_copy(
                            s_["qT"][:, cp_ * 512:(cp_ + 1) * 512], ps[0:E, :])
                        nc.scalar.copy(
                            s_["kT"][:, cp_ * 512:(cp_ + 1) * 512], ps[64:128, :])
                    for cp_ in range(4):
                        ops.append(lambda b=b, cp_=cp_: qk(cp_, b))

                    def uproj(uh, b=b):
                        s_ = st[b]
                        ps = psP.tile([128, 512], F32, tag="pov", bufs=2,
                                      name="ups")
                        for j in range(8):
                            lt = uh * 8 + j
                            nc.tensor.matmul(
                                ps[:, j * 64:(j + 1) * 64],
                                s_["xt"][:, lt * 128:(lt + 1) * 128], wut[:],
                                start=True, stop=True)
                        ua = s_["u_aug"][:].rearrange("p (n e1) -> p n e1", e1=E + 1)
                        if uh == 0:
                            nc.vector.memset(ua[:, :, E:E + 1], 1.0)
                        nc.scalar.copy(
                            ua[:, uh * 8:(uh + 1) * 8, 0:E],
                            ps[:].rearrange("p (n e) -> p n e", e=E))
                    ops.append(lambda b=b: uproj(0, b))
                    ops.append(lambda b=b: uproj(1, b))

                exps_tiles = {}

                def scone(ki, b=b, c=c):
                    s_ = st[b]
                    off = ki - 4 * c
                    q0 = off * 128 if off > 0 else 0
                    ps = psP.tile([128, 512], F32, tag="sc", bufs=2, name="ps")
                    nc.tensor.matmul(
                        ps[:, q0:512],
                        s_["kT"][:, ki * 128:(ki + 1) * 128],
                        s_["qT"][:, c * 512 + q0:(c + 1) * 512],
                        start=True, stop=True)
                    exps = chp.tile([128, 512], BF16, tag="exps", bufs=8,
                                    name="exps")
                    nc.scalar.activation(exps[:, q0:512], ps[:, q0:512], AF.Exp)
                    if off >= 0:
                        # causal triangle on the diagonal 128-col group
                        nc.gpsimd.tensor_tensor(
                            exps[:, q0:q0 + 128], exps[:, q0:q0 + 128],
                            tri[:], MULT)
                    exps_tiles[ki] = exps

                def av(ki, b=b, c=c, nki=nki):
                    if ki == 0:
                        pov = psP.tile([65, 512], F32, tag="pov", bufs=2,
                                       name="pov")
                        st[(b, c, "pov")] = pov
                    pov = st[(b, c, "pov")]
                    eh = exps_tiles.pop(ki)
                    ua = st[b]["u_aug"][:].rearrange(
                        "p (n e1) -> p n e1", e1=E + 1)
                    # diagonal blocks contribute only to q >= off*128
                    off = ki - 4 * c
                    q0 = off * 128 if off > 0 else 0
                    nc.tensor.matmul(
                        pov[:, q0:512], ua[:, ki, :], eh[:, q0:512],
                        start=(ki == 0), stop=(ki == nki - 1),
                        skip_group_check=True)

                # interleave: scores run ~3 blocks ahead of av
                sq = list(range(nki))
                aq = list(range(nki))
                while sq or aq:
                    if sq:
                        ki = sq.pop(0)
                        ops.append(lambda ki=ki: scone(ki))
                    done = nki - len(sq)
                    if aq and (not sq or aq[0] <= done - 3):
                        ki = aq.pop(0)
                        ops.append(lambda ki=ki: av(ki))

                def tail(b=b, c=c):
                    pov = st.pop((b, c, "pov"))
                    o_un = chp.tile([65, 512], BF16, tag="o_un")
                    nc.scalar.copy(o_un[:], pov[:])
                    # stride 66 keeps each PSUM transpose write 4B-aligned
                    tr = psP.tile([128, 4 * 66], BF16, tag="sc", bufs=2)
                    tr_v = tr[:].rearrange("p (n e1) -> p n e1", e1=66)
                    for j in range(4):
                        nc.tensor.transpose(
                            tr_v[:, j, 0:65],
                            o_un[:, j * 128:(j + 1) * 128],
                            ident[0:65, 0:65])
                    rden = chp.tile([128, 4], BF16, tag="rden")
                    with nc.allow_low_precision(reason="bf16 recip of softmax denom, 0.4%"):
                        nc.vector.reciprocal(rden[:], tr_v[:, :, 64])
                    # state_l columns PERMUTED: col p = state e=p+1 (p<63), col 63 = ones
                    state_l = chp.tile([128, 4 * E], BF16, tag="state_l")
                    sl = state_l[:].rearrange("p (n e) -> p n e", e=E)
                    nc.vector.memset(sl[:, :, D:E], 1.0)
                    nc.vector.tensor_tensor(
                        sl[:, :, 0:D], tr_v[:, :, 1:E],
                        rden[:, :, None].to_broadcast([128, 4, D]), MULT)
                    stT_ps = psP.tile([64, 512], BF16, tag="sc", bufs=2)
                    for j in range(4):
                        nc.tensor.transpose(
                            stT_ps[:, j * 128:(j + 1) * 128],
                            sl[:, j, :], ident[:])
                    # stateT slot 0, this batch's half (incl ones row)
                    nc.scalar.copy(
                        states[0][:, b * 512:(b + 1) * 512], stT_ps[:])
                ops.append(tail)
                return ops

            # ================= euler emission =================
            def euler_unit(c, t, bsel=None, msh_on_scalar=True,
                           scalar_quarters=0):
                """One Euler step (states[t] -> states[t+1] + DMA rows 0:63).
                bsel=None: both batches batched; bsel=b: that batch's half
                only (used for t=0 so euler starts right after b's tail)."""
                s_in = states[t]
                s_out = states[t + 1]
                bs = range(BPC) if bsel is None else [bsel]
                w = len(bs) * 512
                x0 = 0 if bsel is None else bsel * 512
                # per (b, half): 4 group matmuls; h layout [128, g*256+l];
                # 4-group product reduce (DVE can read only ONE PSUM operand
                # per instruction, so a tt-tree on h is illegal).
                mm = chp.tile([128, BPC * 512], BF16, tag="mm", bufs=2,
                              name="mm")
                nq = 0
                for b in bs:
                    for half in range(2):
                        h = psP.tile([128, 1024], F32, tag="hh", name="hh",
                                     bufs=2)
                        q0 = b * 512 + half * 256
                        for g in range(4):
                            nc.tensor.matmul(
                                h[:, g * 256:(g + 1) * 256],
                                wall[:, g * 128:(g + 1) * 128],
                                s_in[:, q0:q0 + 256],
                                start=True, stop=True)
                        if nq < scalar_quarters:
                            # scalar moves h to SBUF; DVE tree at 2x beats
                            # the perf-mode-less 1x reduce
                            h_sb = chp.tile([128, 1024], BF16, tag="hsb",
                                            bufs=2, name="hsb")
                            nc.scalar.copy(h_sb[:], h[:])
                            hv = h_sb[:].rearrange(
                                "p (x g l) -> p x g l", x=2, g=2)
                            m1 = chp.tile([128, 512], BF16, tag="m1",
                                          bufs=2, name="m1")
                            m1v = m1[:].rearrange("p (x l) -> p x l", x=2)
                            nc.vector.tensor_tensor(
                                m1v[:], hv[:, :, 0, :], hv[:, :, 1, :], MULT)
                            nc.vector.tensor_tensor(
                                mm[:, q0:q0 + 256],
                                m1v[:, 0, :], m1v[:, 1, :], MULT)
                            nq += 1
                        else:
                            nc.vector.tensor_reduce(
                                mm[:, q0:q0 + 256],
                                h[:].rearrange("p (g l) -> p l g", l=256),
                                mybir.AxisListType.X, MULT)
                # cross-half product + state add (batched over selected bs)
                msh = chp.tile([63, BPC * 512], BF16, tag="msh", bufs=2,
                               name="msh")
                if msh_on_scalar:
                    nc.scalar.copy(msh[:, x0:x0 + w], mm[64:64 + D, x0:x0 + w])
                else:
                    nc.vector.tensor_copy(
                        msh[:, x0:x0 + w], mm[64:64 + D, x0:x0 + w])
                nc.vector.tensor_tensor(
                    msh[:, x0:x0 + w], msh[:, x0:x0 + w],
                    mm[0:D, x0:x0 + w], MULT)
                nc.vector.tensor_tensor(
                    s_out[0:D, x0:x0 + w], s_in[0:D, x0:x0 + w],
                    msh[:, x0:x0 + w], ADD)
                # DMA: src iterates (d, b, q); dst [b, c, t, d, q]
                nc.sync.dma_start(
                    out=bass.AP(
                        tensor=out_e,
                        offset=(c * F_LEN + t) * D * 512
                        + (0 if bsel is None else bsel * NC4 * F_LEN * D * 512),
                        ap=[[512, D], [NC4 * F_LEN * D * 512, len(bs)],
                            [1, 512]]),
                    in_=s_out[0:D, x0:x0 + w].rearrange(
                        "p (b q) -> p b q", b=len(bs)))

            # ================= schedule =================
            def zip_merge(a, b):
                out = []
                for i in range(max(len(a), len(b))):
                    if i < len(a):
                        out.append(a[i])
                    if i < len(b):
                        out.append(b[i])
                return out

            # startup: attention for chunk 0, batches zipped so neither
            # engine chain (sc ring / scalar exp) starves the PE queue
            for op in zip_merge(attn_thunks(0, 0), attn_thunks(1, 0)):
                op()

            # global attention queue with per-chunk completion deadlines:
            # drain as LATE as possible while attn(c) still fully precedes
            # euler(c) -- keeps DVE (euler) busy early, spreads the PE-heavy
            # attention over the whole run.  bounds0[cc] = b0's ops done.
            GQ = []
            bounds = {}
            bounds0 = {}
            for cc in range(1, NC4):
                a0 = attn_thunks(0, cc)
                a1 = attn_thunks(1, cc)
                GQ += zip_merge(a0, a1)
                bounds0[cc] = len(GQ) - (1 if len(a1) >= len(a0) else 0)
                bounds[cc] = len(GQ)
            drained = 0
            units_done = 0
            for c in range(NC4):
                last = c == NC4 - 1
                if c >= 1:
                    # b0's attention done -> start b0's first euler step
                    while drained < bounds0[c]:
                        GQ[drained]()
                        drained += 1
                    euler_unit(c, 0, bsel=0,
                               scalar_quarters=(2 if last else 1 if c == 2 else 0))
                    while drained < bounds[c]:
                        GQ[drained]()
                        drained += 1
                    euler_unit(c, 0, bsel=1,
                               scalar_quarters=(2 if last else 1 if c == 2 else 0))
                else:
                    euler_unit(c, 0)
                units_done += 1
                for t in range(1, F_LEN):
                    euler_unit(c, t,
                               scalar_quarters=(2 if last else 1 if c == 2 else 0))
                    units_done += 1
                    need = 0
                    for cc in range(c + 1, NC4):
                        ub = cc * F_LEN - units_done
                        rem = bounds[cc] - drained
                        if rem <= 0:
                            continue
                        need = max(need, rem if ub <= 0 else -(-rem // ub))
                    for _ in range(need):
                        if drained < len(GQ):
                            GQ[drained]()
                            drained += 1
            while drained < len(GQ):
                GQ[drained]()
                drained += 1

    if split:
        _split_multiwaits(nc)
    return nc


_NC_CACHE = None


def _get_nc():
    global _NC_CACHE
    if _NC_CACHE is None:
        _NC_CACHE = _build_nc()
    return _NC_CACHE


def host_prep(in_proj_w, out_proj_w, Wg, Mg, bg, sigma):
    """Host-side weight prep (shared with the sim test)."""
    bf = mybir.dt.np(BF16)
    s = sigma + EPS
    inv_s_aug = np.concatenate([[1.0], 1.0 / s]).astype(np.float32)
    Win_f = in_proj_w * inv_s_aug[None, :]
    scale = 1.0 / np.sqrt(np.float32(E))
    Wq = Win_f[0:E] * scale
    Wk = Win_f[E:2 * E]
    Wv = Win_f[2 * E:3 * E]

    # fused V*out_proj with physical-units fold
    Wu = out_proj_w @ Wv
    Wu[0, :] = 0.0
    Wu[1:, :] = Wu[1:, :] * s[:, None]

    wqkt = np.concatenate([Wq, Wk], axis=0).T.astype(bf)       # [64, 128]
    wut = Wu.T.astype(bf)                                      # [64, 64]

    # wall grouped: group g is [64, 128]: col d <-> factor 2g ch d, col 64+d
    # <-> factor 2g+1 ch d; cols 63/127 dummy-zero. Rows permuted to the
    # stateT layout (states e=1..63 at partitions 0..62, ones-row at 63).
    Wgm = Wg * Mg
    wall = np.zeros((E, 4 * 128), np.float32)
    for g in range(4):
        for f in range(2):
            j = 2 * g + f
            col = g * 128 + f * 64 + np.arange(D)
            wall[D, col] = Wgm[:, j, 0] + bg[:, j]            # ones-row coeff
            wall[0:D, col] = (Wgm[:, j, 1:] / s[None, :]).T   # states rows
    wall[:, 0:63] *= DT * s[None, :]   # fold DT and s_d into factor 0
    wall = wall.astype(bf)

    kv = np.arange(128)[:, None]
    q = np.arange(128)[None, :]
    tri = (kv <= q).astype(np.float32).astype(bf)
    ident = np.eye(128).astype(bf)
    return wqkt, wut, wall, tri, ident


def kernel(t, inputs, in_proj_w, in_proj_b, out_proj_w, out_proj_b,
           Wg, Mg, bg, sigma):
    inputs = np.asarray(inputs, np.float32)
    in_proj_w = np.asarray(in_proj_w, np.float32)
    in_proj_b = np.asarray(in_proj_b, np.float32)
    out_proj_w = np.asarray(out_proj_w, np.float32)
    out_proj_b = np.asarray(out_proj_b, np.float32)
    Wg = np.asarray(Wg, np.float32)
    Mg = np.asarray(Mg, np.float32)
    bg = np.asarray(bg, np.float32)
    sigma = np.asarray(sigma, np.float32)
    bf = mybir.dt.np(BF16)
    assert np.all(in_proj_b == 0) and np.all(out_proj_b == 0)

    wqkt, wut, wall, tri, ident = host_prep(
        in_proj_w, out_proj_w, Wg, Mg, bg, sigma)

    xt_all = np.ascontiguousarray(
        inputs.reshape(NCORES, BPC, L, E).transpose(0, 1, 3, 2)).astype(bf)

    in_maps = []
    for i in range(NCORES):
        in_maps.append({
            "xt": xt_all[i], "wqkt": wqkt, "wut": wut,
            "wall": wall, "tri": tri, "ident": ident,
        })

    nc = _get_nc()
    res = run_bass_kernel_spmd(nc, in_maps, core_ids=list(range(NCORES)))
    global LAST_RESULTS
    LAST_RESULTS = res
    # out: [BPC, NC4, F_LEN, D, 512] bf16, E-major -> [B, L, F_LEN*D] f32
    outs = []
    for i in range(NCORES):
        o = np.asarray(res.results[i]["out"]).astype(np.float32)
        # [BPC, c, t, d, q] -> [BPC, c, q, t, d]
        o = o.transpose(0, 1, 4, 2, 3).reshape(BPC, L, F_LEN * D)
        outs.append(o)
    return np.ascontiguousarray(np.concatenate(outs, axis=0))


LAST_RESULTS = None


# revision 14
# speedup vs baseline: 1.0988x; 1.0733x over previous
"""Trainium2 Bass kernel v3 for nn_AC_Filter_PreNorm_Net (causal attention +
product-network Euler).

Self-contained: accepts FULL inputs, shards batch over 8 NeuronCores, returns
FULL output.

v3 changes over v2 (251us baseline):
  - Euler 8-factor product as a tensor_tensor tree (tt1 PSUM pair-mult ->
    SBUF bf16, then 4x-mode bf16 tts) instead of tensor_reduce (no DVE perf
    modes) + Pool cross-copy.  Pool msh copy (59.6us) eliminated.
  - Euler state written to a per-step ring of stateT tiles; output DMA'd
    directly from stateT (E-major bf16) and transposed on host.  The 128
    outT transposes + 32 scalar copies are gone.
  - Scores narrowed to the causal trapezoid: diagonal k-tiles only compute
    q >= off*128; single shared [128,128] triangle mask (Pool), zero-fill
    memsets on Pool.
  - pov ring bufs=2 so b1's AV no longer waits b0's tail.
"""
import sys
sys.path.insert(0, "/opt/trn_rl_repo")
import numpy as np
import concourse.bass as bass
import concourse.tile as tile
import bass_rust
from concourse import mybir
from concourse.bass_utils import run_bass_kernel_spmd

F32 = mybir.dt.float32
BF16 = mybir.dt.bfloat16
AF = mybir.ActivationFunctionType
MULT = mybir.AluOpType.mult
ADD = mybir.AluOpType.add

B, L, D = 16, 2048, 63
E = D + 1            # 64
W1 = 8
F_LEN = 4
DT = 0.01
EPS = 1e-5
NCORES = 8
BPC = B // NCORES    # batches per core = 2
NT = L // 128        # l-tiles per batch = 16
NC4 = 4              # q-chunks of 512


def _split_multiwaits(nc):
    """walrus rejects >1 sync wait per instruction; hoist extras onto
    preceding same-engine NoOps."""
    n_added = 0
    for fn in nc.m.functions:
        for bb in fn.blocks:
            insts = list(bb.instructions)
            out = []
            changed = False
            for inst in insts:
                si = inst.sync_info
                if si is not None and si.on_wait is not None and len(si.on_wait) > 1:
                    waits = list(si.on_wait)
                    for w in waits[:-1]:
                        nop = mybir.InstNoOp(
                            name=f"{inst.name}-wsp{n_added}", ins=[], outs=[]
                        )
                        n_added += 1
                        nop.engine = inst.engine
                        nop.sync_info = bass_rust.SyncInfo(on_wait=[w], on_update=[])
                        out.append(nop)
                    si.on_wait = [waits[-1]]
                    changed = True
                out.append(inst)
            if changed:
                bb.instructions = out
    return n_added


def _build_nc(split=True):
    nc = bass.Bass()
    dp = nc.declare_dram_parameter
    xt_e = dp("xt", [BPC, E, L], BF16, isOutput=False)       # host-pretransposed
    wqkt_e = dp("wqkt", [E, 128], BF16, isOutput=False)      # lhsT: [e_in, q|k]
    wut_e = dp("wut", [E, E], BF16, isOutput=False)          # rhs: [e_in, e_out]
    wall_e = dp("wall", [E, 4 * 128], BF16, isOutput=False)  # grouped, DT*s folded
    tri_e = dp("tri", [128, 128], BF16, isOutput=False)      # causal triangle
    ident_e = dp("ident", [128, 128], BF16, isOutput=False)
    import os as _os
    dbg_e = (dp("dbg", [BPC, NC4, 128, 260], F32, isOutput=True)
             if _os.environ.get("DBG_POV") else None)
    # E-major per-step state dump; host transposes to [L, F_LEN*D]
    out_e = dp("out", [BPC, NC4, F_LEN, D, 512], BF16, isOutput=True)

    with tile.TileContext(nc) as tc:
        with (
            tc.tile_pool(name="consts", bufs=1) as cp,
            tc.tile_pool(name="big", bufs=2) as bp,
            tc.tile_pool(name="chk", bufs=2) as chp,
            tc.tile_pool(name="ps", bufs=1, space="PSUM") as psP,
        ):
            # ---- first const (HAM burst + qk gate on it), then inputs ----
            wqkt = cp.tile([E, 128], BF16)
            nc.sync.dma_start(out=wqkt[:], in_=wqkt_e[:])
            xts = []
            for b in range(BPC):
                xt = bp.tile([E, L], BF16, tag="xt")
                for piece in range(4):
                    nc.sync.dma_start(
                        out=xt[:, piece * 512:(piece + 1) * 512],
                        in_=xt_e[b][:, piece * 512:(piece + 1) * 512])
                xts.append(xt)
            wut = cp.tile([E, E], BF16)
            nc.sync.dma_start(out=wut[:], in_=wut_e[:])
            wall = cp.tile([E, 4 * 128], BF16)
            nc.sync.dma_start(out=wall[:], in_=wall_e[:])
            tri = cp.tile([128, 128], BF16)
            nc.sync.dma_start(out=tri[:], in_=tri_e[:])
            ident = cp.tile([128, 128], BF16)
            nc.sync.dma_start(out=ident[:], in_=ident_e[:])

            # activation-table preload (overlaps DMA wait)
            _scr = cp.tile([64, 4], BF16, name="actwarm")
            nc.vector.memset(_scr[:], 0.0)
            nc.scalar.activation(_scr[:], _scr[:], AF.Exp)

            st = {}   # persistent per-batch tiles

            # combined per-step stateT ring [64, b0|b1 512 each]: slot t =
            # state after t Euler steps (slot 0 written by the attention
            # tails each chunk).  Row 63 is the pinned ones-row: slots 1..4
            # only get rows 0:63 written by the Euler add, so set once
            # (whole tile: engine ops need a 0/32/64/96 start partition).
            states = []
            for t in range(F_LEN + 1):
                s_t = cp.tile([E, BPC * 512], BF16, name=f"state{t}")
                states.append(s_t)
                if t > 0:
                    nc.gpsimd.memset(s_t[:], 1.0)

            # ================= attention thunk lists =================
            def attn_thunks(b, c):
                """List of closures emitting attention for (b, c), in
                queue-safe order."""
                ops = []
                nki = 4 * c + 4
                npair = nki // 2

                if c == 0:
                    def ldx(b=b):
                        xt = xts[b]
                        qT = bp.tile([E, L], BF16, tag="qT")
                        kT = bp.tile([E, L], BF16, tag="kT")
                        u_aug = bp.tile([128, NT * (E + 1)], BF16, tag="u_aug")
                        st[b] = {"xt": xt, "qT": qT, "kT": kT, "u_aug": u_aug}
                    ops.append(ldx)

                    def qk(cp_, b=b):
                        s_ = st[b]
                        ps = psP.tile([128, 512], F32, tag="sc", bufs=2,
                                      name="ps")
                        nc.tensor.matmul(
                            ps[:], wqkt[:],
                            s_["xt"][:, cp_ * 512:(cp_ + 1) * 512],
                            start=True, stop=True)
                        nc.vector.tensor_copy(
                            s_["qT"][:, cp_ * 512:(cp_ + 1) * 512], ps[0:E, :])
                        nc.scalar.copy(
                            s_["kT"][:, cp_ * 512:(cp_ + 1) * 512], ps[64:128, :])
                    for cp_ in range(4):
                        ops.append(lambda b=b, cp_=cp_: qk(cp_, b))

                    def uproj(uh, b=b):
                        s_ = st[b]
                        ps = psP.tile([128, 512], F32, tag="pov", bufs=2,
                                      name="ups")
                        for j in range(8):
                            lt = uh * 8 + j
                            nc.tensor.matmul(
                                ps[:, j * 64:(j + 1) * 64],
                                s_["xt"][:, lt * 128:(lt + 1) * 128], wut[:],
                                start=True, stop=True)
                        ua = s_["u_aug"][:].rearrange("p (n e1) -> p n e1", e1=E + 1)
                        if uh == 0:
                            nc.vector.memset(ua[:, :, E:E + 1], 1.0)
                        nc.scalar.copy(
                            ua[:, uh * 8:(uh + 1) * 8, 0:E],
                            ps[:].rearrange("p (n e) -> p n e", e=E))
                    ops.append(lambda b=b: uproj(0, b))
                    ops.append(lambda b=b: uproj(1, b))

                exps_tiles = {}

                def scone(ki, b=b, c=c):
                    s_ = st[b]
                    off = ki - 4 * c
                    q0 = off * 128 if off > 0 else 0
                    ps = psP.tile([128, 512], F32, tag="sc", bufs=2, name="ps")
                    nc.tensor.matmul(
                        ps[:, q0:512],
                        s_["kT"][:, ki * 128:(ki + 1) * 128],
                        s_["qT"][:, c * 512 + q0:(c + 1) * 512],
                        start=True, stop=True)
                    exps = chp.tile([128, 512], BF16, tag="exps", bufs=8,
                                    name="exps")
                    nc.scalar.activation(exps[:, q0:512], ps[:, q0:512], AF.Exp)
                    if off >= 0:
                        # causal triangle on the diagonal 128-col group
                        nc.gpsimd.tensor_tensor(
                            exps[:, q0:q0 + 128], exps[:, q0:q0 + 128],
                            tri[:], MULT)
                    exps_tiles[ki] = exps

                def av(ki, b=b, c=c, nki=nki):
                    if ki == 0:
                        pov = psP.tile([128, 4 * 65], F32, tag="pov", bufs=2,
                                       name="pov")
                        # a start=True on one 65-col region zeroes beyond
                        # it; zero once and accumulate-only instead
                        nc.vector.memset(pov[:], 0.0)
                        st[(b, c, "pov")] = pov
                    pov = st[(b, c, "pov")]
                    eh = exps_tiles.pop(ki)
                    ua = st[b]["u_aug"][:].rearrange(
                        "p (n e1) -> p n e1", e1=E + 1)
                    # transposed AV: exps subtile is the stationary operand,
                    # out povT[q, e] -- 65 moving cols per piece, and the
                    # causal skip (j >= ki-4c) is implicit
                    off = ki - 4 * c
                    for j in range(max(0, off), 4):
                        nc.tensor.matmul(
                            pov[:, j * 65:(j + 1) * 65],
                            eh[:, j * 128:(j + 1) * 128],
                            ua[:, ki, :],
                            start=False, stop=False,
                            skip_group_check=True)

                # interleave: scores run ~3 blocks ahead of av
                sq = list(range(nki))
                aq = list(range(nki))
                while sq or aq:
                    if sq:
                        ki = sq.pop(0)
                        ops.append(lambda ki=ki: scone(ki))
                    done = nki - len(sq)
                    if aq and (not sq or aq[0] <= done - 3):
                        ki = aq.pop(0)
                        ops.append(lambda ki=ki: av(ki))

                def tail(b=b, c=c):
                    pov = st.pop((b, c, "pov"))
                    if dbg_e is not None:
                        dtile = chp.tile([128, 260], F32, tag="dbg")
                        nc.scalar.copy(dtile[:], pov[:])
                        nc.sync.dma_start(out=dbg_e[b, c], in_=dtile[:])
                    pov_v = pov[:].rearrange("p (n e1) -> p n e1", e1=65)
                    rden = chp.tile([128, 4], BF16, tag="rden")
                    with nc.allow_low_precision(reason="bf16 recip of softmax denom, 0.4%"):
                        nc.vector.reciprocal(rden[:], pov_v[:, :, 64])
                    # state_l columns PERMUTED: col p = state e=p+1 (p<63), col 63 = ones
                    state_l = chp.tile([128, 4 * E], BF16, tag="state_l")
                    sl = state_l[:].rearrange("p (n e) -> p n e", e=E)
                    nc.vector.memset(sl[:, :, D:E], 1.0)
                    nc.vector.tensor_tensor(
                        sl[:, :, 0:D], pov_v[:, :, 1:64],
                        rden[:, :, None].to_broadcast([128, 4, D]), MULT)
                    stT_ps = psP.tile([64, 512], BF16, tag="sc", bufs=2)
                    for j in range(4):
                        nc.tensor.transpose(
                            stT_ps[:, j * 128:(j + 1) * 128],
                            sl[:, j, :], ident[:])
                    # stateT slot 0, this batch's half (incl ones row)
                    nc.scalar.copy(
                        states[0][:, b * 512:(b + 1) * 512], stT_ps[:])
                ops.append(tail)
                return ops

            # ================= euler emission =================
            def euler_unit(c, t, bsel=None, msh_on_scalar=True,
                           scalar_quarters=0):
                """One Euler step (states[t] -> states[t+1] + DMA rows 0:63).
                bsel=None: both batches batched; bsel=b: that batch's half
                only (used for t=0 so euler starts right after b's tail)."""
                s_in = states[t]
                s_out = states[t + 1]
                bs = range(BPC) if bsel is None else [bsel]
                w = len(bs) * 512
                x0 = 0 if bsel is None else bsel * 512
                # per (b, half): 4 group matmuls; h layout [128, g*256+l];
                # 4-group product reduce (DVE can read only ONE PSUM operand
                # per instruction, so a tt-tree on h is illegal).
                mm = chp.tile([128, BPC * 512], BF16, tag="mm", bufs=2,
                              name="mm")
                nq = 0
                for b in bs:
                    for half in range(2):
                        h = psP.tile([128, 1024], F32, tag="hh", name="hh",
                                     bufs=2)
                        q0 = b * 512 + half * 256
                        for g in range(4):
                            nc.tensor.matmul(
                                h[:, g * 256:(g + 1) * 256],
                                wall[:, g * 128:(g + 1) * 128],
                                s_in[:, q0:q0 + 256],
                                start=True, stop=True)
                        if nq < scalar_quarters:
                            # scalar moves h to SBUF; DVE tree at 2x beats
                            # the perf-mode-less 1x reduce
                            h_sb = chp.tile([128, 1024], BF16, tag="hsb",
                                            bufs=2, name="hsb")
                            nc.scalar.copy(h_sb[:], h[:])
                            hv = h_sb[:].rearrange(
                                "p (x g l) -> p x g l", x=2, g=2)
                            m1 = chp.tile([128, 512], BF16, tag="m1",
                                          bufs=2, name="m1")
                            m1v = m1[:].rearrange("p (x l) -> p x l", x=2)
                            nc.vector.tensor_tensor(
                                m1v[:], hv[:, :, 0, :], hv[:, :, 1, :], MULT)
                            nc.vector.tensor_tensor(
                                mm[:, q0:q0 + 256],
                                m1v[:, 0, :], m1v[:, 1, :], MULT)
                            nq += 1
                        else:
                            nc.vector.tensor_reduce(
                                mm[:, q0:q0 + 256],
                                h[:].rearrange("p (g l) -> p l g", l=256),
                                mybir.AxisListType.X, MULT)
                # cross-half product + state add (batched over selected bs)
                msh = chp.tile([63, BPC * 512], BF16, tag="msh", bufs=2,
                               name="msh")
                if msh_on_scalar:
                    nc.scalar.copy(msh[:, x0:x0 + w], mm[64:64 + D, x0:x0 + w])
                else:
                    nc.vector.tensor_copy(
                        msh[:, x0:x0 + w], mm[64:64 + D, x0:x0 + w])
                nc.vector.tensor_tensor(
                    msh[:, x0:x0 + w], msh[:, x0:x0 + w],
                    mm[0:D, x0:x0 + w], MULT)
                nc.vector.tensor_tensor(
                    s_out[0:D, x0:x0 + w], s_in[0:D, x0:x0 + w],
                    msh[:, x0:x0 + w], ADD)
                # DMA: src iterates (d, b, q); dst [b, c, t, d, q]
                nc.sync.dma_start(
                    out=bass.AP(
                        tensor=out_e,
                        offset=(c * F_LEN + t) * D * 512
                        + (0 if bsel is None else bsel * NC4 * F_LEN * D * 512),
                        ap=[[512, D], [NC4 * F_LEN * D * 512, len(bs)],
                            [1, 512]]),
                    in_=s_out[0:D, x0:x0 + w].rearrange(
                        "p (b q) -> p b q", b=len(bs)))

            # ================= schedule =================
            def zip_merge(a, b):
                out = []
                for i in range(max(len(a), len(b))):
                    if i < len(a):
                        out.append(a[i])
                    if i < len(b):
                        out.append(b[i])
                return out

            # startup: attention for chunk 0, batches zipped so neither
            # engine chain (sc ring / scalar exp) starves the PE queue
            for op in zip_merge(attn_thunks(0, 0), attn_thunks(1, 0)):
                op()

            # global attention queue with per-chunk completion deadlines:
            # drain as LATE as possible while attn(c) still fully precedes
            # euler(c) -- keeps DVE (euler) busy early, spreads the PE-heavy
            # attention over the whole run.  bounds0[cc] = b0's ops done.
            GQ = []
            bounds = {}
            bounds0 = {}
            for cc in range(1, NC4):
                a0 = attn_thunks(0, cc)
                a1 = attn_thunks(1, cc)
                GQ += zip_merge(a0, a1)
                bounds0[cc] = len(GQ) - (1 if len(a1) >= len(a0) else 0)
                bounds[cc] = len(GQ)
            drained = 0
            units_done = 0
            for c in range(NC4):
                last = c == NC4 - 1
                if c >= 1:
                    # b0's attention done -> start b0's first euler step
                    while drained < bounds0[c]:
                        GQ[drained]()
                        drained += 1
                    euler_unit(c, 0, bsel=0,
                               scalar_quarters=(3 if last else 1 if c == 2 else 0))
                    while drained < bounds[c]:
                        GQ[drained]()
                        drained += 1
                    euler_unit(c, 0, bsel=1,
                               scalar_quarters=(3 if last else 1 if c == 2 else 0))
                else:
                    euler_unit(c, 0)
                units_done += 1
                for t in range(1, F_LEN):
                    euler_unit(c, t,
                               scalar_quarters=(3 if last else 1 if c == 2 else 0))
                    units_done += 1
                    need = 0
                    for cc in range(c + 1, NC4):
                        ub = cc * F_LEN - units_done
                        rem = bounds[cc] - drained
                        if rem <= 0:
                            continue
                        need = max(need, rem if ub <= 0 else -(-rem // ub))
                    for _ in range(need):
                        if drained < len(GQ):
                            GQ[drained]()
                            drained += 1
            while drained < len(GQ):
                GQ[drained]()
                drained += 1

    if split:
        _split_multiwaits(nc)
    return nc


_NC_CACHE = None


def _get_nc():
    global _NC_CACHE
    if _NC_CACHE is None:
        _NC_CACHE = _build_nc()
    return _NC_CACHE


def host_prep(in_proj_w, out_proj_w, Wg, Mg, bg, sigma):
    """Host-side weight prep (shared with the sim test)."""
    bf = mybir.dt.np(BF16)
    s = sigma + EPS
    inv_s_aug = np.concatenate([[1.0], 1.0 / s]).astype(np.float32)
    Win_f = in_proj_w * inv_s_aug[None, :]
    scale = 1.0 / np.sqrt(np.float32(E))
    Wq = Win_f[0:E] * scale
    Wk = Win_f[E:2 * E]
    Wv = Win_f[2 * E:3 * E]

    # fused V*out_proj with physical-units fold
    Wu = out_proj_w @ Wv
    Wu[0, :] = 0.0
    Wu[1:, :] = Wu[1:, :] * s[:, None]

    wqkt = np.concatenate([Wq, Wk], axis=0).T.astype(bf)       # [64, 128]
    wut = Wu.T.astype(bf)                                      # [64, 64]

    # wall grouped: group g is [64, 128]: col d <-> factor 2g ch d, col 64+d
    # <-> factor 2g+1 ch d; cols 63/127 dummy-zero. Rows permuted to the
    # stateT layout (states e=1..63 at partitions 0..62, ones-row at 63).
    Wgm = Wg * Mg
    wall = np.zeros((E, 4 * 128), np.float32)
    for g in range(4):
        for f in range(2):
            j = 2 * g + f
            col = g * 128 + f * 64 + np.arange(D)
            wall[D, col] = Wgm[:, j, 0] + bg[:, j]            # ones-row coeff
            wall[0:D, col] = (Wgm[:, j, 1:] / s[None, :]).T   # states rows
    wall[:, 0:63] *= DT * s[None, :]   # fold DT and s_d into factor 0
    wall = wall.astype(bf)

    kv = np.arange(128)[:, None]
    q = np.arange(128)[None, :]
    tri = (kv <= q).astype(np.float32).astype(bf)
    ident = np.eye(128).astype(bf)
    return wqkt, wut, wall, tri, ident


def kernel(t, inputs, in_proj_w, in_proj_b, out_proj_w, out_proj_b,
           Wg, Mg, bg, sigma):
    inputs = np.asarray(inputs, np.float32)
    in_proj_w = np.asarray(in_proj_w, np.float32)
    in_proj_b = np.asarray(in_proj_b, np.float32)
    out_proj_w = np.asarray(out_proj_w, np.float32)
    out_proj_b = np.asarray(out_proj_b, np.float32)
    Wg = np.asarray(Wg, np.float32)
    Mg = np.asarray(Mg, np.float32)
    bg = np.asarray(bg, np.float32)
    sigma = np.asarray(sigma, np.float32)
    bf = mybir.dt.np(BF16)
    assert np.all(in_proj_b == 0) and np.all(out_proj_b == 0)

    wqkt, wut, wall, tri, ident = host_prep(
        in_proj_w, out_proj_w, Wg, Mg, bg, sigma)

    xt_all = np.ascontiguousarray(
        inputs.reshape(NCORES, BPC, L, E).transpose(0, 1, 3, 2)).astype(bf)

    in_maps = []
    for i in range(NCORES):
        in_maps.append({
            "xt": xt_all[i], "wqkt": wqkt, "wut": wut,
            "wall": wall, "tri": tri, "ident": ident,
        })

    nc = _get_nc()
    res = run_bass_kernel_spmd(nc, in_maps, core_ids=list(range(NCORES)))
    global LAST_RESULTS
    LAST_RESULTS = res
    # out: [BPC, NC4, F_LEN, D, 512] bf16, E-major -> [B, L, F_LEN*D] f32
    outs = []
    for i in range(NCORES):
        o = np.asarray(res.results[i]["out"]).astype(np.float32)
        # [BPC, c, t, d, q] -> [BPC, c, q, t, d]
        o = o.transpose(0, 1, 4, 2, 3).reshape(BPC, L, F_LEN * D)
        outs.append(o)
    return np.ascontiguousarray(np.concatenate(outs, axis=0))


LAST_RESULTS = None


# revision 15
# speedup vs baseline: 1.1038x; 1.0045x over previous
"""Trainium2 Bass kernel v3 for nn_AC_Filter_PreNorm_Net (causal attention +
product-network Euler).

Self-contained: accepts FULL inputs, shards batch over 8 NeuronCores, returns
FULL output.

v3 changes over v2 (251us baseline):
  - Euler 8-factor product as a tensor_tensor tree (tt1 PSUM pair-mult ->
    SBUF bf16, then 4x-mode bf16 tts) instead of tensor_reduce (no DVE perf
    modes) + Pool cross-copy.  Pool msh copy (59.6us) eliminated.
  - Euler state written to a per-step ring of stateT tiles; output DMA'd
    directly from stateT (E-major bf16) and transposed on host.  The 128
    outT transposes + 32 scalar copies are gone.
  - Scores narrowed to the causal trapezoid: diagonal k-tiles only compute
    q >= off*128; single shared [128,128] triangle mask (Pool), zero-fill
    memsets on Pool.
  - pov ring bufs=2 so b1's AV no longer waits b0's tail.
"""
import sys
sys.path.insert(0, "/opt/trn_rl_repo")
import numpy as np
import concourse.bass as bass
import concourse.tile as tile
import bass_rust
from concourse import mybir
from concourse.bass_utils import run_bass_kernel_spmd

F32 = mybir.dt.float32
BF16 = mybir.dt.bfloat16
AF = mybir.ActivationFunctionType
MULT = mybir.AluOpType.mult
ADD = mybir.AluOpType.add

B, L, D = 16, 2048, 63
E = D + 1            # 64
W1 = 8
F_LEN = 4
DT = 0.01
EPS = 1e-5
NCORES = 8
BPC = B // NCORES    # batches per core = 2
NT = L // 128        # l-tiles per batch = 16
NC4 = 4              # q-chunks of 512


def _split_multiwaits(nc):
    """walrus rejects >1 sync wait per instruction; hoist extras onto
    preceding same-engine NoOps."""
    n_added = 0
    for fn in nc.m.functions:
        for bb in fn.blocks:
            insts = list(bb.instructions)
            out = []
            changed = False
            for inst in insts:
                si = inst.sync_info
                if si is not None and si.on_wait is not None and len(si.on_wait) > 1:
                    waits = list(si.on_wait)
                    for w in waits[:-1]:
                        nop = mybir.InstNoOp(
                            name=f"{inst.name}-wsp{n_added}", ins=[], outs=[]
                        )
                        n_added += 1
                        nop.engine = inst.engine
                        nop.sync_info = bass_rust.SyncInfo(on_wait=[w], on_update=[])
                        out.append(nop)
                    si.on_wait = [waits[-1]]
                    changed = True
                out.append(inst)
            if changed:
                bb.instructions = out
    return n_added


def _build_nc(split=True):
    nc = bass.Bass()
    dp = nc.declare_dram_parameter
    xt_e = dp("xt", [BPC, E, L], BF16, isOutput=False)       # host-pretransposed
    wqkt_e = dp("wqkt", [E, 128], BF16, isOutput=False)      # lhsT: [e_in, q|k]
    wut_e = dp("wut", [E, E], BF16, isOutput=False)          # rhs: [e_in, e_out]
    wall_e = dp("wall", [E, 4 * 128], BF16, isOutput=False)  # grouped, DT*s folded
    tri_e = dp("tri", [128, 128], BF16, isOutput=False)      # causal triangle
    ident_e = dp("ident", [128, 128], BF16, isOutput=False)
    import os as _os
    dbg_e = (dp("dbg", [BPC, NC4, 128, 260], F32, isOutput=True)
             if _os.environ.get("DBG_POV") else None)
    # E-major per-step state dump; host transposes to [L, F_LEN*D]
    out_e = dp("out", [BPC, NC4, F_LEN, D, 512], BF16, isOutput=True)

    with tile.TileContext(nc) as tc:
        with (
            tc.tile_pool(name="consts", bufs=1) as cp,
            tc.tile_pool(name="big", bufs=2) as bp,
            tc.tile_pool(name="chk", bufs=2) as chp,
            tc.tile_pool(name="ps", bufs=1, space="PSUM") as psP,
        ):
            # ---- first const (HAM burst + qk gate on it), then inputs ----
            wqkt = cp.tile([E, 128], BF16)
            nc.sync.dma_start(out=wqkt[:], in_=wqkt_e[:])
            xts = []
            for b in range(BPC):
                xt = bp.tile([E, L], BF16, tag="xt")
                for piece in range(4):
                    nc.sync.dma_start(
                        out=xt[:, piece * 512:(piece + 1) * 512],
                        in_=xt_e[b][:, piece * 512:(piece + 1) * 512])
                xts.append(xt)
            wut = cp.tile([E, E], BF16)
            nc.sync.dma_start(out=wut[:], in_=wut_e[:])
            wall = cp.tile([E, 4 * 128], BF16)
            nc.sync.dma_start(out=wall[:], in_=wall_e[:])
            tri = cp.tile([128, 128], BF16)
            nc.sync.dma_start(out=tri[:], in_=tri_e[:])
            ident = cp.tile([128, 128], BF16)
            nc.sync.dma_start(out=ident[:], in_=ident_e[:])

            # activation-table preload (overlaps DMA wait)
            _scr = cp.tile([64, 4], BF16, name="actwarm")
            nc.vector.memset(_scr[:], 0.0)
            nc.scalar.activation(_scr[:], _scr[:], AF.Exp)

            st = {}   # persistent per-batch tiles

            # combined per-step stateT ring [64, b0|b1 512 each]: slot t =
            # state after t Euler steps (slot 0 written by the attention
            # tails each chunk).  Row 63 is the pinned ones-row: slots 1..4
            # only get rows 0:63 written by the Euler add, so set once
            # (whole tile: engine ops need a 0/32/64/96 start partition).
            states = []
            for t in range(F_LEN + 1):
                s_t = cp.tile([E, BPC * 512], BF16, name=f"state{t}")
                states.append(s_t)
                if t > 0:
                    nc.gpsimd.memset(s_t[:], 1.0)

            # ================= attention thunk lists =================
            def attn_thunks(b, c):
                """List of closures emitting attention for (b, c), in
                queue-safe order."""
                ops = []
                nki = 4 * c + 4
                npair = nki // 2

                if c == 0:
                    def ldx(b=b):
                        xt = xts[b]
                        qT = bp.tile([E, L], BF16, tag="qT")
                        kT = bp.tile([E, L], BF16, tag="kT")
                        u_aug = bp.tile([128, NT * (E + 1)], BF16, tag="u_aug")
                        st[b] = {"xt": xt, "qT": qT, "kT": kT, "u_aug": u_aug}
                    ops.append(ldx)

                    def qk(cp_, b=b):
                        s_ = st[b]
                        ps = psP.tile([128, 512], F32, tag="sc", bufs=2,
                                      name="ps")
                        nc.tensor.matmul(
                            ps[:], wqkt[:],
                            s_["xt"][:, cp_ * 512:(cp_ + 1) * 512],
                            start=True, stop=True)
                        nc.vector.tensor_copy(
                            s_["qT"][:, cp_ * 512:(cp_ + 1) * 512], ps[0:E, :])
                        nc.scalar.copy(
                            s_["kT"][:, cp_ * 512:(cp_ + 1) * 512], ps[64:128, :])
                    for cp_ in range(4):
                        ops.append(lambda b=b, cp_=cp_: qk(cp_, b))

                    def uproj(uh, b=b):
                        s_ = st[b]
                        ps = psP.tile([128, 512], F32, tag="pov", bufs=2,
                                      name="ups")
                        for j in range(8):
                            lt = uh * 8 + j
                            nc.tensor.matmul(
                                ps[:, j * 64:(j + 1) * 64],
                                s_["xt"][:, lt * 128:(lt + 1) * 128], wut[:],
                                start=True, stop=True)
                        ua = s_["u_aug"][:].rearrange("p (n e1) -> p n e1", e1=E + 1)
                        if uh == 0:
                            nc.vector.memset(ua[:, :, E:E + 1], 1.0)
                        nc.scalar.copy(
                            ua[:, uh * 8:(uh + 1) * 8, 0:E],
                            ps[:].rearrange("p (n e) -> p n e", e=E))
                    ops.append(lambda b=b: uproj(0, b))
                    ops.append(lambda b=b: uproj(1, b))

                exps_tiles = {}

                def scone(ki, b=b, c=c):
                    s_ = st[b]
                    off = ki - 4 * c
                    q0 = off * 128 if off > 0 else 0
                    ps = psP.tile([128, 512], F32, tag="sc", bufs=2, name="ps")
                    nc.tensor.matmul(
                        ps[:, q0:512],
                        s_["kT"][:, ki * 128:(ki + 1) * 128],
                        s_["qT"][:, c * 512 + q0:(c + 1) * 512],
                        start=True, stop=True)
                    exps = chp.tile([128, 512], BF16, tag="exps", bufs=12,
                                    name="exps")
                    nc.scalar.activation(exps[:, q0:512], ps[:, q0:512], AF.Exp)
                    if off >= 0:
                        # causal triangle on the diagonal 128-col group
                        nc.gpsimd.tensor_tensor(
                            exps[:, q0:q0 + 128], exps[:, q0:q0 + 128],
                            tri[:], MULT)
                    exps_tiles[ki] = exps

                def av(ki, b=b, c=c, nki=nki):
                    if ki == 0:
                        pov = psP.tile([128, 4 * 65], F32, tag="pov", bufs=2,
                                       name="pov")
                        # a start=True on one 65-col region zeroes beyond
                        # it; zero once and accumulate-only instead
                        nc.vector.memset(pov[:], 0.0)
                        st[(b, c, "pov")] = pov
                    pov = st[(b, c, "pov")]
                    eh = exps_tiles.pop(ki)
                    ua = st[b]["u_aug"][:].rearrange(
                        "p (n e1) -> p n e1", e1=E + 1)
                    # transposed AV: exps subtile is the stationary operand,
                    # out povT[q, e] -- 65 moving cols per piece, and the
                    # causal skip (j >= ki-4c) is implicit
                    off = ki - 4 * c
                    for j in range(max(0, off), 4):
                        nc.tensor.matmul(
                            pov[:, j * 65:(j + 1) * 65],
                            eh[:, j * 128:(j + 1) * 128],
                            ua[:, ki, :],
                            start=False, stop=False,
                            skip_group_check=True)

                # interleave: scores run ~3 blocks ahead of av
                sq = list(range(nki))
                aq = list(range(nki))
                while sq or aq:
                    if sq:
                        ki = sq.pop(0)
                        ops.append(lambda ki=ki: scone(ki))
                    done = nki - len(sq)
                    if aq and (not sq or aq[0] <= done - 3):
                        ki = aq.pop(0)
                        ops.append(lambda ki=ki: av(ki))

                def tail(b=b, c=c):
                    pov = st.pop((b, c, "pov"))
                    if dbg_e is not None:
                        dtile = chp.tile([128, 260], F32, tag="dbg")
                        nc.scalar.copy(dtile[:], pov[:])
                        nc.sync.dma_start(out=dbg_e[b, c], in_=dtile[:])
                    pov_v = pov[:].rearrange("p (n e1) -> p n e1", e1=65)
                    rden = chp.tile([128, 4], BF16, tag="rden")
                    with nc.allow_low_precision(reason="bf16 recip of softmax denom, 0.4%"):
                        nc.vector.reciprocal(rden[:], pov_v[:, :, 64])
                    # state_l columns PERMUTED: col p = state e=p+1 (p<63), col 63 = ones
                    state_l = chp.tile([128, 4 * E], BF16, tag="state_l")
                    sl = state_l[:].rearrange("p (n e) -> p n e", e=E)
                    nc.vector.memset(sl[:, :, D:E], 1.0)
                    nc.vector.tensor_tensor(
                        sl[:, :, 0:D], pov_v[:, :, 1:64],
                        rden[:, :, None].to_broadcast([128, 4, D]), MULT)
                    stT_ps = psP.tile([64, 512], BF16, tag="sc", bufs=2)
                    for j in range(4):
                        nc.tensor.transpose(
                            stT_ps[:, j * 128:(j + 1) * 128],
                            sl[:, j, :], ident[:])
                    # stateT slot 0, this batch's half (incl ones row)
                    nc.scalar.copy(
                        states[0][:, b * 512:(b + 1) * 512], stT_ps[:])
                ops.append(tail)
                return ops

            # ================= euler emission =================
            def euler_unit(c, t, bsel=None, msh_on_scalar=True,
                           scalar_quarters=0):
                """One Euler step (states[t] -> states[t+1] + DMA rows 0:63).
                bsel=None: both batches batched; bsel=b: that batch's half
                only (used for t=0 so euler starts right after b's tail)."""
                s_in = states[t]
                s_out = states[t + 1]
                bs = range(BPC) if bsel is None else [bsel]
                w = len(bs) * 512
                x0 = 0 if bsel is None else bsel * 512
                # per (b, half): 4 group matmuls; h layout [128, g*256+l];
                # 4-group product reduce (DVE can read only ONE PSUM operand
                # per instruction, so a tt-tree on h is illegal).
                mm = chp.tile([128, BPC * 512], BF16, tag="mm", bufs=3,
                              name="mm")
                nq = 0
                for b in bs:
                    for half in range(2):
                        h = psP.tile([128, 1024], F32, tag="hh", name="hh",
                                     bufs=2)
                        q0 = b * 512 + half * 256
                        for g in range(4):
                            nc.tensor.matmul(
                                h[:, g * 256:(g + 1) * 256],
                                wall[:, g * 128:(g + 1) * 128],
                                s_in[:, q0:q0 + 256],
                                start=True, stop=True)
                        if nq < scalar_quarters:
                            # scalar moves h to SBUF; DVE tree at 2x beats
                            # the perf-mode-less 1x reduce
                            h_sb = chp.tile([128, 1024], BF16, tag="hsb",
                                            bufs=3, name="hsb")
                            nc.scalar.copy(h_sb[:], h[:])
                            hv = h_sb[:].rearrange(
                                "p (x g l) -> p x g l", x=2, g=2)
                            m1 = chp.tile([128, 512], BF16, tag="m1",
                                          bufs=3, name="m1")
                            m1v = m1[:].rearrange("p (x l) -> p x l", x=2)
                            nc.vector.tensor_tensor(
                                m1v[:], hv[:, :, 0, :], hv[:, :, 1, :], MULT)
                            nc.vector.tensor_tensor(
                                mm[:, q0:q0 + 256],
                                m1v[:, 0, :], m1v[:, 1, :], MULT)
                            nq += 1
                        else:
                            nc.vector.tensor_reduce(
                                mm[:, q0:q0 + 256],
                                h[:].rearrange("p (g l) -> p l g", l=256),
                                mybir.AxisListType.X, MULT)
                # cross-half product + state add (batched over selected bs)
                msh = chp.tile([63, BPC * 512], BF16, tag="msh", bufs=3,
                               name="msh")
                if msh_on_scalar:
                    nc.scalar.copy(msh[:, x0:x0 + w], mm[64:64 + D, x0:x0 + w])
                else:
                    nc.vector.tensor_copy(
                        msh[:, x0:x0 + w], mm[64:64 + D, x0:x0 + w])
                nc.vector.tensor_tensor(
                    msh[:, x0:x0 + w], msh[:, x0:x0 + w],
                    mm[0:D, x0:x0 + w], MULT)
                nc.vector.tensor_tensor(
                    s_out[0:D, x0:x0 + w], s_in[0:D, x0:x0 + w],
                    msh[:, x0:x0 + w], ADD)
                # DMA: src iterates (d, b, q); dst [b, c, t, d, q]
                nc.sync.dma_start(
                    out=bass.AP(
                        tensor=out_e,
                        offset=(c * F_LEN + t) * D * 512
                        + (0 if bsel is None else bsel * NC4 * F_LEN * D * 512),
                        ap=[[512, D], [NC4 * F_LEN * D * 512, len(bs)],
                            [1, 512]]),
                    in_=s_out[0:D, x0:x0 + w].rearrange(
                        "p (b q) -> p b q", b=len(bs)))

            # ================= schedule =================
            def zip_merge(a, b):
                out = []
                for i in range(max(len(a), len(b))):
                    if i < len(a):
                        out.append(a[i])
                    if i < len(b):
                        out.append(b[i])
                return out

            # startup: attention for chunk 0, batches zipped so neither
            # engine chain (sc ring / scalar exp) starves the PE queue
            for op in zip_merge(attn_thunks(0, 0), attn_thunks(1, 0)):
                op()

            # global attention queue with per-chunk completion deadlines:
            # drain as LATE as possible while attn(c) still fully precedes
            # euler(c) -- keeps DVE (euler) busy early, spreads the PE-heavy
            # attention over the whole run.  bounds0[cc] = b0's ops done.
            GQ = []
            bounds = {}
            bounds0 = {}
            for cc in range(1, NC4):
                a0 = attn_thunks(0, cc)
                a1 = attn_thunks(1, cc)
                GQ += zip_merge(a0, a1)
                bounds0[cc] = len(GQ) - (1 if len(a1) >= len(a0) else 0)
                bounds[cc] = len(GQ)
            drained = 0
            units_done = 0
            for c in range(NC4):
                last = c == NC4 - 1
                if c >= 1:
                    # b0's attention done -> start b0's first euler step
                    while drained < bounds0[c]:
                        GQ[drained]()
                        drained += 1
                    euler_unit(c, 0, bsel=0,
                               scalar_quarters=(3 if last else 2 if c == 2 else 0))
                    while drained < bounds[c]:
                        GQ[drained]()
                        drained += 1
                    euler_unit(c, 0, bsel=1,
                               scalar_quarters=(3 if last else 2 if c == 2 else 0))
                else:
                    euler_unit(c, 0)
                units_done += 1
                for t in range(1, F_LEN):
                    euler_unit(c, t,
                               scalar_quarters=(3 if last else 2 if c == 2 else 0))
                    units_done += 1
                    need = 0
                    for cc in range(c + 1, NC4):
                        ub = cc * F_LEN - units_done
                        rem = bounds[cc] - drained
                        if rem <= 0:
                            continue
                        need = max(need, rem if ub <= 0 else -(-rem // ub))
                    for _ in range(need):
                        if drained < len(GQ):
                            GQ[drained]()
                            drained += 1
            while drained < len(GQ):
                GQ[drained]()
                drained += 1

    if split:
        _split_multiwaits(nc)
    return nc


_NC_CACHE = None


def _get_nc():
    global _NC_CACHE
    if _NC_CACHE is None:
        _NC_CACHE = _build_nc()
    return _NC_CACHE


def host_prep(in_proj_w, out_proj_w, Wg, Mg, bg, sigma):
    """Host-side weight prep (shared with the sim test)."""
    bf = mybir.dt.np(BF16)
    s = sigma + EPS
    inv_s_aug = np.concatenate([[1.0], 1.0 / s]).astype(np.float32)
    Win_f = in_proj_w * inv_s_aug[None, :]
    scale = 1.0 / np.sqrt(np.float32(E))
    Wq = Win_f[0:E] * scale
    Wk = Win_f[E:2 * E]
    Wv = Win_f[2 * E:3 * E]

    # fused V*out_proj with physical-units fold
    Wu = out_proj_w @ Wv
    Wu[0, :] = 0.0
    Wu[1:, :] = Wu[1:, :] * s[:, None]

    wqkt = np.concatenate([Wq, Wk], axis=0).T.astype(bf)       # [64, 128]
    wut = Wu.T.astype(bf)                                      # [64, 64]

    # wall grouped: group g is [64, 128]: col d <-> factor 2g ch d, col 64+d
    # <-> factor 2g+1 ch d; cols 63/127 dummy-zero. Rows permuted to the
    # stateT layout (states e=1..63 at partitions 0..62, ones-row at 63).
    Wgm = Wg * Mg
    wall = np.zeros((E, 4 * 128), np.float32)
    for g in range(4):
        for f in range(2):
            j = 2 * g + f
            col = g * 128 + f * 64 + np.arange(D)
            wall[D, col] = Wgm[:, j, 0] + bg[:, j]            # ones-row coeff
            wall[0:D, col] = (Wgm[:, j, 1:] / s[None, :]).T   # states rows
    wall[:, 0:63] *= DT * s[None, :]   # fold DT and s_d into factor 0
    wall = wall.astype(bf)

    kv = np.arange(128)[:, None]
    q = np.arange(128)[None, :]
    tri = (kv <= q).astype(np.float32).astype(bf)
    ident = np.eye(128).astype(bf)
    return wqkt, wut, wall, tri, ident


def kernel(t, inputs, in_proj_w, in_proj_b, out_proj_w, out_proj_b,
           Wg, Mg, bg, sigma):
    inputs = np.asarray(inputs, np.float32)
    in_proj_w = np.asarray(in_proj_w, np.float32)
    in_proj_b = np.asarray(in_proj_b, np.float32)
    out_proj_w = np.asarray(out_proj_w, np.float32)
    out_proj_b = np.asarray(out_proj_b, np.float32)
    Wg = np.asarray(Wg, np.float32)
    Mg = np.asarray(Mg, np.float32)
    bg = np.asarray(bg, np.float32)
    sigma = np.asarray(sigma, np.float32)
    bf = mybir.dt.np(BF16)
    assert np.all(in_proj_b == 0) and np.all(out_proj_b == 0)

    wqkt, wut, wall, tri, ident = host_prep(
        in_proj_w, out_proj_w, Wg, Mg, bg, sigma)

    xt_all = np.ascontiguousarray(
        inputs.reshape(NCORES, BPC, L, E).transpose(0, 1, 3, 2)).astype(bf)

    in_maps = []
    for i in range(NCORES):
        in_maps.append({
            "xt": xt_all[i], "wqkt": wqkt, "wut": wut,
            "wall": wall, "tri": tri, "ident": ident,
        })

    nc = _get_nc()
    res = run_bass_kernel_spmd(nc, in_maps, core_ids=list(range(NCORES)))
    global LAST_RESULTS
    LAST_RESULTS = res
    # out: [BPC, NC4, F_LEN, D, 512] bf16, E-major -> [B, L, F_LEN*D] f32
    outs = []
    for i in range(NCORES):
        o = np.asarray(res.results[i]["out"]).astype(np.float32)
        # [BPC, c, t, d, q] -> [BPC, c, q, t, d]
        o = o.transpose(0, 1, 4, 2, 3).reshape(BPC, L, F_LEN * D)
        outs.append(o)
    return np.ascontiguousarray(np.concatenate(outs, axis=0))


LAST_RESULTS = None


# revision 16
# speedup vs baseline: 1.1464x; 1.0386x over previous
"""Trainium2 Bass kernel v3 for nn_AC_Filter_PreNorm_Net (causal attention +
product-network Euler).

Self-contained: accepts FULL inputs, shards batch over 8 NeuronCores, returns
FULL output.

v3 changes over v2 (251us baseline):
  - Euler 8-factor product as a tensor_tensor tree (tt1 PSUM pair-mult ->
    SBUF bf16, then 4x-mode bf16 tts) instead of tensor_reduce (no DVE perf
    modes) + Pool cross-copy.  Pool msh copy (59.6us) eliminated.
  - Euler state written to a per-step ring of stateT tiles; output DMA'd
    directly from stateT (E-major bf16) and transposed on host.  The 128
    outT transposes + 32 scalar copies are gone.
  - Scores narrowed to the causal trapezoid: diagonal k-tiles only compute
    q >= off*128; single shared [128,128] triangle mask (Pool), zero-fill
    memsets on Pool.
  - pov ring bufs=2 so b1's AV no longer waits b0's tail.
"""
import sys
sys.path.insert(0, "/opt/trn_rl_repo")
import numpy as np
import concourse.bass as bass
import concourse.tile as tile
import bass_rust
from concourse import mybir
from concourse.bass_utils import run_bass_kernel_spmd

F32 = mybir.dt.float32
BF16 = mybir.dt.bfloat16
AF = mybir.ActivationFunctionType
MULT = mybir.AluOpType.mult
ADD = mybir.AluOpType.add

B, L, D = 16, 2048, 63
E = D + 1            # 64
W1 = 8
F_LEN = 4
DT = 0.01
EPS = 1e-5
NCORES = 8
BPC = B // NCORES    # batches per core = 2
NT = L // 128        # l-tiles per batch = 16
NC4 = 4              # q-chunks of 512


def _split_multiwaits(nc):
    """walrus rejects >1 sync wait per instruction; hoist extras onto
    preceding same-engine NoOps."""
    n_added = 0
    for fn in nc.m.functions:
        for bb in fn.blocks:
            insts = list(bb.instructions)
            out = []
            changed = False
            for inst in insts:
                si = inst.sync_info
                if si is not None and si.on_wait is not None and len(si.on_wait) > 1:
                    waits = list(si.on_wait)
                    for w in waits[:-1]:
                        nop = mybir.InstNoOp(
                            name=f"{inst.name}-wsp{n_added}", ins=[], outs=[]
                        )
                        n_added += 1
                        nop.engine = inst.engine
                        nop.sync_info = bass_rust.SyncInfo(on_wait=[w], on_update=[])
                        out.append(nop)
                    si.on_wait = [waits[-1]]
                    changed = True
                out.append(inst)
            if changed:
                bb.instructions = out
    return n_added


def _build_nc(split=True):
    nc = bass.Bass()
    dp = nc.declare_dram_parameter
    xt_e = dp("xt", [BPC, E, L], BF16, isOutput=False)       # host-pretransposed
    wqkt_e = dp("wqkt", [E, 128], BF16, isOutput=False)      # lhsT: [e_in, q|k]
    wut_e = dp("wut", [E, E], BF16, isOutput=False)          # rhs: [e_in, e_out]
    wall_e = dp("wall", [E, 4 * 128], BF16, isOutput=False)  # grouped, DT*s folded
    tri_e = dp("tri", [128, 128], BF16, isOutput=False)      # causal triangle
    ident_e = dp("ident", [128, 128], BF16, isOutput=False)
    import os as _os
    dbg_e = (dp("dbg", [BPC, NC4, 128, 260], F32, isOutput=True)
             if _os.environ.get("DBG_POV") else None)
    # E-major per-step state dump; host transposes to [L, F_LEN*D]
    out_e = dp("out", [BPC, NC4, F_LEN, D, 512], BF16, isOutput=True)

    with tile.TileContext(nc) as tc:
        with (
            tc.tile_pool(name="consts", bufs=1) as cp,
            tc.tile_pool(name="big", bufs=2) as bp,
            tc.tile_pool(name="chk", bufs=2) as chp,
            tc.tile_pool(name="ps", bufs=1, space="PSUM") as psP,
        ):
            # ---- first const (HAM burst + qk gate on it), then inputs ----
            wqkt = cp.tile([E, 128], BF16)
            nc.sync.dma_start(out=wqkt[:], in_=wqkt_e[:])
            xts = []
            for b in range(BPC):
                xt = bp.tile([E, L], BF16, tag="xt")
                for piece in range(4):
                    nc.sync.dma_start(
                        out=xt[:, piece * 512:(piece + 1) * 512],
                        in_=xt_e[b][:, piece * 512:(piece + 1) * 512])
                xts.append(xt)
            wut = cp.tile([E, E], BF16)
            nc.sync.dma_start(out=wut[:], in_=wut_e[:])
            wall = cp.tile([E, 4 * 128], BF16)
            nc.sync.dma_start(out=wall[:], in_=wall_e[:])
            tri = cp.tile([128, 128], BF16)
            nc.sync.dma_start(out=tri[:], in_=tri_e[:])
            ident = cp.tile([128, 128], BF16)
            nc.sync.dma_start(out=ident[:], in_=ident_e[:])

            # activation-table preload (overlaps DMA wait)
            _scr = cp.tile([64, 4], BF16, name="actwarm")
            nc.vector.memset(_scr[:], 0.0)
            nc.scalar.activation(_scr[:], _scr[:], AF.Exp)

            st = {}   # persistent per-batch tiles

            # combined per-step stateT ring [64, b0|b1 512 each]: slot t =
            # state after t Euler steps (slot 0 written by the attention
            # tails each chunk).  Row 63 is the pinned ones-row: slots 1..4
            # only get rows 0:63 written by the Euler add, so set once
            # (whole tile: engine ops need a 0/32/64/96 start partition).
            rings = []
            for r in range(2):
                ring = []
                for t in range(F_LEN + 1):
                    s_t = cp.tile([E, BPC * 512], BF16, name=f"state{r}_{t}")
                    ring.append(s_t)
                    if t > 0:
                        nc.gpsimd.memset(s_t[:], 1.0)
                rings.append(ring)

            # ================= attention thunk lists =================
            def attn_thunks(b, c):
                """List of closures emitting attention for (b, c), in
                queue-safe order."""
                ops = []
                nki = 4 * c + 4
                npair = nki // 2

                if c == 0:
                    def ldx(b=b):
                        xt = xts[b]
                        qT = bp.tile([E, L], BF16, tag="qT")
                        kT = bp.tile([E, L], BF16, tag="kT")
                        u_aug = bp.tile([128, NT * (E + 1)], BF16, tag="u_aug")
                        st[b] = {"xt": xt, "qT": qT, "kT": kT, "u_aug": u_aug}
                    ops.append(ldx)

                    def qk(cp_, b=b):
                        s_ = st[b]
                        ps = psP.tile([128, 512], F32, tag="sc", bufs=2,
                                      name="ps")
                        nc.tensor.matmul(
                            ps[:], wqkt[:],
                            s_["xt"][:, cp_ * 512:(cp_ + 1) * 512],
                            start=True, stop=True)
                        nc.vector.tensor_copy(
                            s_["qT"][:, cp_ * 512:(cp_ + 1) * 512], ps[0:E, :])
                        nc.scalar.copy(
                            s_["kT"][:, cp_ * 512:(cp_ + 1) * 512], ps[64:128, :])
                    for cp_ in range(4):
                        ops.append(lambda b=b, cp_=cp_: qk(cp_, b))

                    def uproj(uh, b=b):
                        s_ = st[b]
                        ps = psP.tile([128, 512], F32, tag="pov", bufs=2,
                                      name="ups")
                        for j in range(8):
                            lt = uh * 8 + j
                            nc.tensor.matmul(
                                ps[:, j * 64:(j + 1) * 64],
                                s_["xt"][:, lt * 128:(lt + 1) * 128], wut[:],
                                start=True, stop=True)
                        ua = s_["u_aug"][:].rearrange("p (n e1) -> p n e1", e1=E + 1)
                        if uh == 0:
                            nc.vector.memset(ua[:, :, E:E + 1], 1.0)
                        nc.scalar.copy(
                            ua[:, uh * 8:(uh + 1) * 8, 0:E],
                            ps[:].rearrange("p (n e) -> p n e", e=E))
                    ops.append(lambda b=b: uproj(0, b))
                    ops.append(lambda b=b: uproj(1, b))

                exps_tiles = {}

                def scone(ki, b=b, c=c):
                    s_ = st[b]
                    off = ki - 4 * c
                    q0 = off * 128 if off > 0 else 0
                    ps = psP.tile([128, 512], F32, tag="sc", bufs=2, name="ps")
                    nc.tensor.matmul(
                        ps[:, q0:512],
                        s_["kT"][:, ki * 128:(ki + 1) * 128],
                        s_["qT"][:, c * 512 + q0:(c + 1) * 512],
                        start=True, stop=True)
                    exps = chp.tile([128, 512], BF16, tag="exps", bufs=12,
                                    name="exps")
                    nc.scalar.activation(exps[:, q0:512], ps[:, q0:512], AF.Exp)
                    if off >= 0:
                        # causal triangle on the diagonal 128-col group
                        nc.gpsimd.tensor_tensor(
                            exps[:, q0:q0 + 128], exps[:, q0:q0 + 128],
                            tri[:], MULT)
                    exps_tiles[ki] = exps

                def av(ki, b=b, c=c, nki=nki):
                    if ki == 0:
                        pov = psP.tile([128, 4 * 65], F32, tag="pov", bufs=2,
                                       name="pov")
                        # a start=True on one 65-col region zeroes beyond
                        # it; zero once and accumulate-only instead
                        nc.vector.memset(pov[:], 0.0)
                        st[(b, c, "pov")] = pov
                    pov = st[(b, c, "pov")]
                    eh = exps_tiles.pop(ki)
                    ua = st[b]["u_aug"][:].rearrange(
                        "p (n e1) -> p n e1", e1=E + 1)
                    # transposed AV: exps subtile is the stationary operand,
                    # out povT[q, e] -- 65 moving cols per piece, and the
                    # causal skip (j >= ki-4c) is implicit
                    off = ki - 4 * c
                    for j in range(max(0, off), 4):
                        nc.tensor.matmul(
                            pov[:, j * 65:(j + 1) * 65],
                            eh[:, j * 128:(j + 1) * 128],
                            ua[:, ki, :],
                            start=False, stop=False,
                            skip_group_check=True)

                # interleave: scores run ~3 blocks ahead of av
                sq = list(range(nki))
                aq = list(range(nki))
                while sq or aq:
                    if sq:
                        ki = sq.pop(0)
                        ops.append(lambda ki=ki: scone(ki))
                    done = nki - len(sq)
                    if aq and (not sq or aq[0] <= done - 3):
                        ki = aq.pop(0)
                        ops.append(lambda ki=ki: av(ki))

                def tail(b=b, c=c):
                    pov = st.pop((b, c, "pov"))
                    if dbg_e is not None:
                        dtile = chp.tile([128, 260], F32, tag="dbg")
                        nc.scalar.copy(dtile[:], pov[:])
                        nc.sync.dma_start(out=dbg_e[b, c], in_=dtile[:])
                    pov_v = pov[:].rearrange("p (n e1) -> p n e1", e1=65)
                    rden = chp.tile([128, 4], BF16, tag="rden")
                    with nc.allow_low_precision(reason="bf16 recip of softmax denom, 0.4%"):
                        nc.vector.reciprocal(rden[:], pov_v[:, :, 64])
                    # state_l columns PERMUTED: col p = state e=p+1 (p<63), col 63 = ones
                    state_l = chp.tile([128, 4 * E], BF16, tag="state_l")
                    sl = state_l[:].rearrange("p (n e) -> p n e", e=E)
                    nc.vector.memset(sl[:, :, D:E], 1.0)
                    nc.vector.tensor_tensor(
                        sl[:, :, 0:D], pov_v[:, :, 1:64],
                        rden[:, :, None].to_broadcast([128, 4, D]), MULT)
                    stT_ps = psP.tile([64, 512], BF16, tag="sc", bufs=2)
                    for j in range(4):
                        nc.tensor.transpose(
                            stT_ps[:, j * 128:(j + 1) * 128],
                            sl[:, j, :], ident[:])
                    # stateT slot 0, this batch's half (incl ones row)
                    nc.scalar.copy(
                        rings[c % 2][0][:, b * 512:(b + 1) * 512], stT_ps[:])
                ops.append(tail)
                return ops

            # ================= euler emission =================
            def euler_unit(c, t, bsel=None, msh_on_scalar=True,
                           scalar_quarters=0):
                """One Euler step (states[t] -> states[t+1] + DMA rows 0:63).
                bsel=None: both batches batched; bsel=b: that batch's half
                only (used for t=0 so euler starts right after b's tail)."""
                s_in = rings[c % 2][t]
                s_out = rings[c % 2][t + 1]
                bs = range(BPC) if bsel is None else [bsel]
                w = len(bs) * 512
                x0 = 0 if bsel is None else bsel * 512
                # per (b, half): 4 group matmuls; h layout [128, g*256+l];
                # 4-group product reduce (DVE can read only ONE PSUM operand
                # per instruction, so a tt-tree on h is illegal).
                mm = chp.tile([128, BPC * 512], BF16, tag="mm", bufs=3,
                              name="mm")
                nq = 0
                for b in bs:
                    for half in range(2):
                        h = psP.tile([128, 1024], F32, tag="hh", name="hh",
                                     bufs=2)
                        q0 = b * 512 + half * 256
                        for g in range(4):
                            nc.tensor.matmul(
                                h[:, g * 256:(g + 1) * 256],
                                wall[:, g * 128:(g + 1) * 128],
                                s_in[:, q0:q0 + 256],
                                start=True, stop=True)
                        if nq < scalar_quarters:
                            # scalar moves h to SBUF; DVE tree at 2x beats
                            # the perf-mode-less 1x reduce
                            h_sb = chp.tile([128, 1024], BF16, tag="hsb",
                                            bufs=3, name="hsb")
                            nc.scalar.copy(h_sb[:], h[:])
                            hv = h_sb[:].rearrange(
                                "p (x g l) -> p x g l", x=2, g=2)
                            m1 = chp.tile([128, 512], BF16, tag="m1",
                                          bufs=3, name="m1")
                            m1v = m1[:].rearrange("p (x l) -> p x l", x=2)
                            nc.vector.tensor_tensor(
                                m1v[:], hv[:, :, 0, :], hv[:, :, 1, :], MULT)
                            nc.vector.tensor_tensor(
                                mm[:, q0:q0 + 256],
                                m1v[:, 0, :], m1v[:, 1, :], MULT)
                            nq += 1
                        else:
                            nc.vector.tensor_reduce(
                                mm[:, q0:q0 + 256],
                                h[:].rearrange("p (g l) -> p l g", l=256),
                                mybir.AxisListType.X, MULT)
                # cross-half product + state add (batched over selected bs)
                msh = chp.tile([63, BPC * 512], BF16, tag="msh", bufs=3,
                               name="msh")
                if msh_on_scalar:
                    nc.scalar.copy(msh[:, x0:x0 + w], mm[64:64 + D, x0:x0 + w])
                else:
                    nc.vector.tensor_copy(
                        msh[:, x0:x0 + w], mm[64:64 + D, x0:x0 + w])
                nc.vector.tensor_tensor(
                    msh[:, x0:x0 + w], msh[:, x0:x0 + w],
                    mm[0:D, x0:x0 + w], MULT)
                nc.vector.tensor_tensor(
                    s_out[0:D, x0:x0 + w], s_in[0:D, x0:x0 + w],
                    msh[:, x0:x0 + w], ADD)
                # DMA: src iterates (d, b, q); dst [b, c, t, d, q]
                nc.sync.dma_start(
                    out=bass.AP(
                        tensor=out_e,
                        offset=(c * F_LEN + t) * D * 512
                        + (0 if bsel is None else bsel * NC4 * F_LEN * D * 512),
                        ap=[[512, D], [NC4 * F_LEN * D * 512, len(bs)],
                            [1, 512]]),
                    in_=s_out[0:D, x0:x0 + w].rearrange(
                        "p (b q) -> p b q", b=len(bs)))

            # ================= schedule =================
            def zip_merge(a, b):
                out = []
                for i in range(max(len(a), len(b))):
                    if i < len(a):
                        out.append(a[i])
                    if i < len(b):
                        out.append(b[i])
                return out

            # global attention queue with per-chunk completion deadlines.
            # attn(c1) must land by unit 4, attn(c2) AND attn(c3) by unit 8:
            # the euler of the last two chunks runs as one merged phase of
            # two interleaved chains (separate state rings), so sem latency
            # hides behind the sibling chain instead of idling every engine.
            GQ = []
            bounds = {}
            bounds0 = {}
            for cc in range(1, NC4):
                a0 = attn_thunks(0, cc)
                a1 = attn_thunks(1, cc)
                GQ += zip_merge(a0, a1)
                bounds0[cc] = len(GQ) - (1 if len(a1) >= len(a0) else 0)
                bounds[cc] = len(GQ)
            drained = 0

            # startup: attention for chunk 0, batches zipped; once both
            # batches' qk/uproj are emitted, trickle early GQ scones in to
            # keep the PE queue dense (stop before GQ's first av -- its pov
            # ring slot would stall the queue on chunk 0's tail)
            su = zip_merge(attn_thunks(0, 0), attn_thunks(1, 0))
            for i, op in enumerate(su):
                op()
                if i >= 10 and drained < 6:
                    GQ[drained]()
                    drained += 1

            units_done = 0
            for c in range(2):
                if c == 1:
                    # b0's attention done -> start b0's first euler step
                    while drained < bounds0[1]:
                        GQ[drained]()
                        drained += 1
                    euler_unit(1, 0, bsel=0)
                    while drained < bounds[1]:
                        GQ[drained]()
                        drained += 1
                    euler_unit(1, 0, bsel=1)
                else:
                    euler_unit(0, 0)
                units_done += 1
                for t in range(1, F_LEN):
                    euler_unit(c, t)
                    units_done += 1
                    need = 0
                    for cc in range(c + 1, NC4):
                        ub = min(cc, 2) * F_LEN - units_done
                        rem = bounds[cc] - drained
                        if rem <= 0:
                            continue
                        need = max(need, rem if ub <= 0 else -(-rem // ub))
                    for _ in range(need):
                        if drained < len(GQ):
                            GQ[drained]()
                            drained += 1
            # force-complete all remaining attention, then the merged
            # two-chain euler of chunks 2 and 3
            while drained < len(GQ):
                GQ[drained]()
                drained += 1
            for t in range(F_LEN):
                euler_unit(2, t, scalar_quarters=2)
                euler_unit(3, t, scalar_quarters=3)

    if split:
        _split_multiwaits(nc)
    return nc


_NC_CACHE = None


def _get_nc():
    global _NC_CACHE
    if _NC_CACHE is None:
        _NC_CACHE = _build_nc()
    return _NC_CACHE


def host_prep(in_proj_w, out_proj_w, Wg, Mg, bg, sigma):
    """Host-side weight prep (shared with the sim test)."""
    bf = mybir.dt.np(BF16)
    s = sigma + EPS
    inv_s_aug = np.concatenate([[1.0], 1.0 / s]).astype(np.float32)
    Win_f = in_proj_w * inv_s_aug[None, :]
    scale = 1.0 / np.sqrt(np.float32(E))
    Wq = Win_f[0:E] * scale
    Wk = Win_f[E:2 * E]
    Wv = Win_f[2 * E:3 * E]

    # fused V*out_proj with physical-units fold
    Wu = out_proj_w @ Wv
    Wu[0, :] = 0.0
    Wu[1:, :] = Wu[1:, :] * s[:, None]

    wqkt = np.concatenate([Wq, Wk], axis=0).T.astype(bf)       # [64, 128]
    wut = Wu.T.astype(bf)                                      # [64, 64]

    # wall grouped: group g is [64, 128]: col d <-> factor 2g ch d, col 64+d
    # <-> factor 2g+1 ch d; cols 63/127 dummy-zero. Rows permuted to the
    # stateT layout (states e=1..63 at partitions 0..62, ones-row at 63).
    Wgm = Wg * Mg
    wall = np.zeros((E, 4 * 128), np.float32)
    for g in range(4):
        for f in range(2):
            j = 2 * g + f
            col = g * 128 + f * 64 + np.arange(D)
            wall[D, col] = Wgm[:, j, 0] + bg[:, j]            # ones-row coeff
            wall[0:D, col] = (Wgm[:, j, 1:] / s[None, :]).T   # states rows
    wall[:, 0:63] *= DT * s[None, :]   # fold DT and s_d into factor 0
    wall = wall.astype(bf)

    kv = np.arange(128)[:, None]
    q = np.arange(128)[None, :]
    tri = (kv <= q).astype(np.float32).astype(bf)
    ident = np.eye(128).astype(bf)
    return wqkt, wut, wall, tri, ident


def kernel(t, inputs, in_proj_w, in_proj_b, out_proj_w, out_proj_b,
           Wg, Mg, bg, sigma):
    inputs = np.asarray(inputs, np.float32)
    in_proj_w = np.asarray(in_proj_w, np.float32)
    in_proj_b = np.asarray(in_proj_b, np.float32)
    out_proj_w = np.asarray(out_proj_w, np.float32)
    out_proj_b = np.asarray(out_proj_b, np.float32)
    Wg = np.asarray(Wg, np.float32)
    Mg = np.asarray(Mg, np.float32)
    bg = np.asarray(bg, np.float32)
    sigma = np.asarray(sigma, np.float32)
    bf = mybir.dt.np(BF16)
    assert np.all(in_proj_b == 0) and np.all(out_proj_b == 0)

    wqkt, wut, wall, tri, ident = host_prep(
        in_proj_w, out_proj_w, Wg, Mg, bg, sigma)

    xt_all = np.ascontiguousarray(
        inputs.reshape(NCORES, BPC, L, E).transpose(0, 1, 3, 2)).astype(bf)

    in_maps = []
    for i in range(NCORES):
        in_maps.append({
            "xt": xt_all[i], "wqkt": wqkt, "wut": wut,
            "wall": wall, "tri": tri, "ident": ident,
        })

    nc = _get_nc()
    res = run_bass_kernel_spmd(nc, in_maps, core_ids=list(range(NCORES)))
    global LAST_RESULTS
    LAST_RESULTS = res
    # out: [BPC, NC4, F_LEN, D, 512] bf16, E-major -> [B, L, F_LEN*D] f32
    outs = []
    for i in range(NCORES):
        o = np.asarray(res.results[i]["out"]).astype(np.float32)
        # [BPC, c, t, d, q] -> [BPC, c, q, t, d]
        o = o.transpose(0, 1, 4, 2, 3).reshape(BPC, L, F_LEN * D)
        outs.append(o)
    return np.ascontiguousarray(np.concatenate(outs, axis=0))


LAST_RESULTS = None
